# revision 1
# baseline (speedup 1.0000x reference)
# Trainium2 Bass kernel for per-sample channel-attention module (CAM).
#
# Reference math per sample (C=512, N=H*W=4096):
#   X = x.reshape(C, N)
#   phi = Wp X ; theta = Wt X ; g = Wg X
#   attn = softmax_rows(phi @ theta^T)          # [C, C]
#   y = attn @ g                                 # [C, N]
#   Z = (y^T).flatten().reshape(C, N)            # torch permute+view reinterpretation
#   out = gamma * (Wm @ Z) + x
#
# Algebraic restructuring (cuts PE work ~1.8x vs the naive 6-GEMM chain):
#   G = X X^T                  (Gram, [C, C])
#   L = Wp G Wt^T              (attention logits via two small GEMMs)
#   A' = softmax(L) @ Wg       (fold g-projection into attention)
#   y = A' X                   (single big GEMM)
# The torch permute+view reinterpretation is free: y^T blocks are produced
# with a stride-8 column selection of X as the stationary matmul operand, so
# each PSUM tile lands exactly on a contiguous block of Z's SBUF layout.
#
# All matmuls run in float32r (fp32 data streamed through the PE at
# 1 row/cycle; operands carry 11 explicit mantissa bits, RNE). The host
# pre-rounds inputs to the fp32r grid and also pre-computes pure layout
# transforms: X^T, Wp^T, Wt^T, and gamma*Wm^T (zero FLOPs of the reference
# are moved off-device; every GEMM/softmax runs on the NeuronCore).

import os
import numpy as np

import concourse.bass as bass
import concourse.mybir as mybir
import concourse.tile as tile
from concourse import bacc
from concourse.bass_utils import run_bass_kernel_spmd
from concourse.tile import TileContext
from concourse.masks import make_identity

P = 128          # partitions
C = 512          # channels
N = 4096         # spatial (64*64)
CC = C // P      # 4 channel chunks
NT = N // P      # 32 spatial tiles
QF = N // C      # 8 fold factor for the permute+view reinterpretation
FP32 = mybir.dt.float32
FP32R = mybir.dt.float32r


def _f32(ap):
    # reinterpret an fp32r tile as plain fp32 (identical bit layout)
    return ap.bitcast(FP32)


def _build_nc():
    nc = bacc.Bacc("TRN2", target_bir_lowering=False, debug=False, num_devices=8)
    x_d = nc.dram_tensor("x", [C, N], FP32R, kind="ExternalInput").ap()
    xt_d = nc.dram_tensor("xt", [N, C], FP32R, kind="ExternalInput").ap()
    wphiT_d = nc.dram_tensor("w_phi_t", [C, C], FP32R, kind="ExternalInput").ap()
    wthetaT_d = nc.dram_tensor("w_theta_t", [C, C], FP32R, kind="ExternalInput").ap()
    wg_d = nc.dram_tensor("w_g", [C, C], FP32R, kind="ExternalInput").ap()
    wmTg_d = nc.dram_tensor("w_mask_t_g", [C, C], FP32R, kind="ExternalInput").ap()
    out_d = nc.dram_tensor("out", [C, N], FP32, kind="ExternalOutput").ap()

    with TileContext(nc) as tc:
        _body(tc, x_d, xt_d, wphiT_d, wthetaT_d, wg_d, wmTg_d, out_d)
    nc.compile()
    return nc


def _body(tc, x_d, xt_d, wphiT_d, wthetaT_d, wg_d, wmTg_d, out_d):
    nc = tc.nc
    from contextlib import ExitStack

    with ExitStack() as ctx:
        const = ctx.enter_context(tc.tile_pool(name="const", bufs=1))
        xpool = ctx.enter_context(tc.tile_pool(name="xpool", bufs=1))
        wpool = ctx.enter_context(tc.tile_pool(name="wpool", bufs=1))
        bigpool = ctx.enter_context(tc.tile_pool(name="bigpool", bufs=1))
        scratch = ctx.enter_context(tc.tile_pool(name="scratch", bufs=2))
        vecs = ctx.enter_context(tc.tile_pool(name="vecs", bufs=8))
        outp = ctx.enter_context(tc.tile_pool(name="outp", bufs=6))
        ps = ctx.enter_context(tc.tile_pool(name="ps", bufs=4, space="PSUM"))
        psg = ctx.enter_context(tc.tile_pool(name="psg", bufs=4, space="PSUM"))

        identity = const.tile([P, P], FP32)
        make_identity(nc, identity)

        # ~12 throwaway matmuls warm the PE (HAM un-throttles after ~3.4 us
        # of activity) while the first xt chunk is still in flight.
        warm = psg.tile([P, P], FP32, tag="gacc")
        for _ in range(12):
            nc.tensor.matmul(warm, identity, identity, start=True, stop=True)

        # ---- weight loads on the SWDGE path (gpsimd) so they never delay the
        # latency-critical xt stream on the Sync HWDGE queue.
        # Layout [p, cc, j]: tile[p, cc, j] = W[128*cc + p, j].
        wphiT = wpool.tile([P, CC, C], FP32R)
        wthetaT = wpool.tile([P, CC, C], FP32R)
        wg_sb = wpool.tile([P, CC, C], FP32R)
        wmT = wpool.tile([P, CC, C], FP32R)
        # Weight + x loads go on the ACT HWDGE queue (nc.scalar.dma_start)
        # so the latency-critical xt stream owns the Sync queue. Order by
        # first use: wthetaT (T1), wphiT (L), wg (A'), then x, then wmT.
        # x arrives in column-quarters: ZS pass ci only reads columns
        # [1024*ci, 1024*(ci+1)), so quarter ci unblocks that pass.
        x_sb = xpool.tile([P, CC, N], FP32R)
        QW = N // CC  # 1024

        def _load_x_quarter(ci):
            nc.scalar.dma_start(
                out=x_sb[:, :, ci * QW:(ci + 1) * QW],
                in_=x_d[:, ci * QW:(ci + 1) * QW].rearrange(
                    "(cc p) n -> p cc n", p=P
                ),
            )

        nc.scalar.dma_start(
            out=wthetaT, in_=wthetaT_d.rearrange("(cc p) j -> p cc j", p=P)
        )
        nc.scalar.dma_start(
            out=wphiT, in_=wphiT_d.rearrange("(cc p) j -> p cc j", p=P)
        )
        _load_x_quarter(0)
        nc.scalar.dma_start(
            out=wg_sb, in_=wg_d.rearrange("(cc p) j -> p cc j", p=P)
        )
        _load_x_quarter(1)
        _load_x_quarter(2)
        _load_x_quarter(3)
        nc.scalar.dma_start(
            out=wmT, in_=wmTg_d.rearrange("(cc p) j -> p cc j", p=P)
        )

        # ---- stream X^T in 1 MB chunks (4 tiles each) and fold each tile
        # into the Gram accumulators as soon as its chunk lands.
        # XT[p, t, c] = X[c, 128*t + p];  G[a, b] = sum_n X[a, n] X[b, n].
        xt_sb = bigpool.tile([P, NT, C], FP32R, tag="big")
        gacc = [
            psg.tile([P, C], FP32, tag="gacc", name=f"gacc{i}")
            for i in range(CC)
        ]
        # Ramped chunk sizes: small first chunks start the Gram stream ~2 us
        # earlier; steady-state 4-tile (1 MB) chunks keep issue overhead low.
        chunks = [2, 2, 4, 4, 4, 4, 4, 4, 4]
        t0c = 0
        for csz in chunks:
            nc.sync.dma_start(
                out=xt_sb[:, t0c:t0c + csz, :],
                in_=xt_d[t0c * P:(t0c + csz) * P, :].rearrange(
                    "(tt p) c -> p tt c", p=P
                ),
            )
            for k in range(csz):
                t = t0c + k
                for mc in range(CC):
                    nc.tensor.matmul(
                        gacc[mc],
                        xt_sb[:, t, mc * P:(mc + 1) * P],
                        xt_sb[:, t, :],
                        start=(t == 0),
                        stop=(t == NT - 1),
                    )
            t0c += csz
        assert t0c == NT

        g_sb = scratch.tile([P, CC, C], FP32R, tag="s8")
        for mc in range(CC):
            nc.any.tensor_copy(g_sb[:, mc, :], gacc[mc])

        # ---- T1 = G @ Wt^T  (uses G symmetry for the stationary operand)
        t1_sb = scratch.tile([P, CC, C], FP32R, tag="s8")
        for mc in range(CC):
            tp = ps.tile([P, C], FP32, tag="ps")
            for jc in range(CC):
                nc.tensor.matmul(
                    tp,
                    g_sb[:, jc, mc * P:(mc + 1) * P],
                    wthetaT[:, jc, :],
                    start=(jc == 0),
                    stop=(jc == CC - 1),
                )
            nc.any.tensor_copy(t1_sb[:, mc, :], tp)

        # ---- L = Wp @ T1 ; softmax rows -> attn
        attn_sb = scratch.tile([P, CC, C], FP32R, tag="s8")
        for mc in range(CC):
            lp = ps.tile([P, C], FP32, tag="ps")
            for ic in range(CC):
                nc.tensor.matmul(
                    lp,
                    wphiT[:, ic, mc * P:(mc + 1) * P],
                    t1_sb[:, ic, :],
                    start=(ic == 0),
                    stop=(ic == CC - 1),
                )
            neg_max = vecs.tile([P, 1], FP32)
            nc.vector.tensor_reduce(
                out=neg_max, in_=lp, axis=mybir.AxisListType.X,
                op=mybir.AluOpType.max, negate=True,
            )
            sums = vecs.tile([P, 1], FP32)
            nc.scalar.activation(
                out=attn_sb[:, mc, :], in_=lp,
                func=mybir.ActivationFunctionType.Exp,
                bias=neg_max, scale=1.0, accum_out=sums,
            )
            rinv = vecs.tile([P, 1], FP32)
            nc.vector.reciprocal(rinv, sums)
            nc.vector.tensor_scalar_mul(
                attn_sb[:, mc, :], attn_sb[:, mc, :], rinv
            )

        # ---- attn^T via PE transposes (fp32 mode; copies round to fp32r)
        attnT_sb = scratch.tile([P, CC, C], FP32R, tag="s8")
        for dc in range(CC):
            pt = ps.tile([P, C], FP32, tag="ps")
            for mc in range(CC):
                nc.tensor.transpose(
                    pt[:, mc * P:(mc + 1) * P],
                    _f32(attn_sb[:, mc, dc * P:(dc + 1) * P]),
                    identity,
                )
            nc.any.tensor_copy(attnT_sb[:, dc, :], pt)

        # ---- A'^T[j, c] = sum_d Wg[d, j] attn[c, d]
        apT_sb = scratch.tile([P, CC, C], FP32R, tag="s8")
        for jc in range(CC):
            ap_ps = ps.tile([P, C], FP32, tag="ps")
            for dc in range(CC):
                nc.tensor.matmul(
                    ap_ps,
                    wg_sb[:, dc, jc * P:(jc + 1) * P],
                    attnT_sb[:, dc, :],
                    start=(dc == 0),
                    stop=(dc == CC - 1),
                )
            nc.any.tensor_copy(apT_sb[:, jc, :], ap_ps)

        # ---- y^T blocks straight into Z layout, interleaved q-major with the
        # final mask GEMM + residual + store.
        # Z[i, q*512 + r] = y^T[8*i + q, r]; with n = 1024*ci + 8*m + q the
        # output PSUM tile [m, r] equals ZS[:, ci, q*512:(q+1)*512], and the
        # mask GEMM for output block jb=q only needs ZS blocks (ci=0..3, q).
        zs_sb = bigpool.tile([P, CC, N], FP32R, tag="big")
        for ci in range(CC):
            for q in range(QF):
                zp = ps.tile([P, C], FP32, tag="ps")
                for jc in range(CC):
                    xr = x_sb[:, jc, :].rearrange(
                        "p (ci m q) -> p ci q m", ci=CC, q=QF
                    )
                    nc.tensor.matmul(
                        zp,
                        xr[:, ci, q, :],
                        apT_sb[:, jc, :],
                        start=(jc == 0),
                        stop=(jc == CC - 1),
                    )
                nc.any.tensor_copy(zs_sb[:, ci, q * C:(q + 1) * C], zp)

                if ci == CC - 1:
                    # ZS blocks (0..3, q) are now all done: emit output block q
                    jb = q
                    for oc in range(CC):
                        mp = psg.tile([P, C], FP32, tag="gacc")
                        for ic in range(CC):
                            nc.tensor.matmul(
                                mp,
                                wmT[:, ic, oc * P:(oc + 1) * P],
                                zs_sb[:, ic, jb * C:(jb + 1) * C],
                                start=(ic == 0),
                                stop=(ic == CC - 1),
                            )
                        ot = outp.tile([P, C], FP32)
                        nc.vector.tensor_add(
                            ot, mp, _f32(x_sb[:, oc, jb * C:(jb + 1) * C])
                        )
                        nc.sync.dma_start(
                            out=out_d[oc * P:(oc + 1) * P, jb * C:(jb + 1) * C],
                            in_=ot,
                        )


_NC_CACHE = {}
LAST_RESULT = None


def get_nc():
    if "nc" not in _NC_CACHE:
        _NC_CACHE["nc"] = _build_nc()
    return _NC_CACHE["nc"]


def _round_fp32r(x):
    """Round fp32 array to the fp32r grid (11 explicit mantissa bits, RNE)."""
    u = np.ascontiguousarray(x, dtype=np.float32).view(np.uint32).astype(np.uint64)
    shift = 23 - 11
    add = (np.uint64(1) << np.uint64(shift - 1)) - np.uint64(1) + (
        (u >> np.uint64(shift)) & np.uint64(1)
    )
    u = (u + add) & np.uint64(~((1 << shift) - 1) & 0xFFFFFFFF)
    return u.astype(np.uint32).view(np.float32)


def make_in_map(xb, w_phi_t, w_theta_t, w_g, w_mask_t_g):
    """Per-core input dict; xb is one sample [C, H, W]."""
    xr = _round_fp32r(xb.reshape(C, N))
    return {
        "x": xr,
        "xt": np.ascontiguousarray(xr.T),
        "w_phi_t": w_phi_t,
        "w_theta_t": w_theta_t,
        "w_g": w_g,
        "w_mask_t_g": w_mask_t_g,
    }


def prep_weights(w_phi, w_theta, w_g, w_mask, gamma):
    w_phi_t = _round_fp32r(np.asarray(w_phi, dtype=np.float32).T)
    w_theta_t = _round_fp32r(np.asarray(w_theta, dtype=np.float32).T)
    w_g_r = _round_fp32r(np.asarray(w_g, dtype=np.float32))
    gamma64 = float(np.asarray(gamma, dtype=np.float32).reshape(-1)[0])
    w_mask_t_g = _round_fp32r(
        (np.asarray(w_mask, dtype=np.float64).T * gamma64).astype(np.float32)
    )
    return w_phi_t, w_theta_t, w_g_r, w_mask_t_g


def kernel(x, w_phi, w_theta, w_g, w_mask, gamma):
    global LAST_RESULT
    x = np.ascontiguousarray(np.asarray(x, dtype=np.float32))
    B, c, h, w = x.shape
    assert (c, h * w) == (C, N), (x.shape,)

    w_phi_t, w_theta_t, w_g_r, w_mask_t_g = prep_weights(
        w_phi, w_theta, w_g, w_mask, gamma
    )
    nc = get_nc()
    in_maps = [
        make_in_map(x[b], w_phi_t, w_theta_t, w_g_r, w_mask_t_g)
        for b in range(B)
    ]
    trace = bool(int(os.environ.get("KERNEL_TRACE", "0")))
    res = run_bass_kernel_spmd(nc, in_maps, list(range(B)), trace=trace)
    LAST_RESULT = res
    out = np.stack([res.results[b]["out"].reshape(c, h, w) for b in range(B)])
    return out



# revision 3
# speedup vs baseline: 1.4448x; 1.4448x over previous
# Trainium2 Bass kernel for per-sample channel-attention module (CAM).
#
# Reference math per sample (C=512, N=H*W=4096):
#   X = x.reshape(C, N)
#   phi = Wp X ; theta = Wt X ; g = Wg X
#   attn = softmax_rows(phi @ theta^T)          # [C, C]
#   y = attn @ g                                 # [C, N]
#   Z = (y^T).flatten().reshape(C, N)            # torch permute+view reinterpretation
#   out = gamma * (Wm @ Z) + x
#
# Algebraic restructuring (cuts PE work ~1.8x vs the naive 6-GEMM chain):
#   G = X X^T                  (Gram, [C, C])
#   L = Wp G Wt^T              (attention logits via two small GEMMs)
#   A' = softmax(L) @ Wg       (fold g-projection into attention)
#   y = A' X                   (single big GEMM)
# The torch permute+view reinterpretation is free: y^T blocks are produced
# with a stride-8 column selection of X as the stationary matmul operand, so
# each PSUM tile lands exactly on a contiguous block of Z's SBUF layout.
#
# Mixed precision (validated vs the fp64 reference; softmax here is a hard
# argmax with top1-top2 logit gaps ~O(10-100), so post-softmax stages are
# linear in quantization error while pre-softmax logits need ~11 bits):
#   - X^T stream + Gram matmuls: fp16 (halves the dominant DMA stream)
#   - G, T1, Wp, Wt: fp32r (logit path)
#   - attn, Wg: fp16 / fp32 softmax
#   - A', X, Z, gamma*Wm^T: fp8 e4m3 with power-of-2 scales; the ZS and mask
#     GEMMs run in DoubleRow perf mode (2 k-tiles per pass, 0.5 cyc/row = 4x
#     the fp32r rate)
#   - residual: fp16 x pre-loaded, added via PSUM prefill (scale 2^16) so the
#     final PSUM->SBUF copy is a single scaled cast to the fp16 output
# Gram exploits symmetry: only the upper-triangle blocks are computed
# (1280 of 2048 moving columns per tile), the 6 lower blocks come from PE
# transposes.

import os
import numpy as np
import ml_dtypes

import concourse.bass as bass
import concourse.mybir as mybir
import concourse.tile as tile
from concourse import bacc
from concourse.bass_utils import run_bass_kernel_spmd
from concourse.tile import TileContext
from concourse.masks import make_identity

P = 128          # partitions
C = 512          # channels
N = 4096         # spatial (64*64)
CC = C // P      # 4 channel chunks
NT = N // P      # 32 spatial tiles
QF = N // C      # 8 fold factor for the permute+view reinterpretation
FP32 = mybir.dt.float32
FP32R = mybir.dt.float32r
FP16 = mybir.dt.float16
FP8 = mybir.dt.float8e4

S_X = 8.0        # x fp8 scale
S_A = 512.0      # A' fp8 scale
S_M = 8192.0     # gamma*Wm^T fp8 scale
S_MX = S_M * S_X           # 65536: residual prefill scale
ZCAST = S_X / (S_A * S_X)  # PSUM (y*S_A*S_X) -> Z*S_X


def _f32(ap):
    # reinterpret an fp32r tile as plain fp32 (identical bit layout)
    return ap.bitcast(FP32)


def _build_nc():
    nc = bacc.Bacc("TRN2", target_bir_lowering=False, debug=False, num_devices=8)
    xt_d = nc.dram_tensor("xt16", [N, C], FP16, kind="ExternalInput").ap()
    x16_d = nc.dram_tensor("x16", [C, N], FP16, kind="ExternalInput").ap()
    x8_d = nc.dram_tensor("x8", [C, N], FP8, kind="ExternalInput").ap()
    wphiT_d = nc.dram_tensor("w_phi_t", [C, C], FP32R, kind="ExternalInput").ap()
    wthetaT_d = nc.dram_tensor("w_theta_t", [C, C], FP32R, kind="ExternalInput").ap()
    wg_d = nc.dram_tensor("w_g16", [C, C], FP16, kind="ExternalInput").ap()
    wmT8_d = nc.dram_tensor("w_mask_t8", [C, C], FP8, kind="ExternalInput").ap()
    out_d = nc.dram_tensor("out", [C, N], FP16, kind="ExternalOutput").ap()

    with TileContext(nc) as tc:
        _body(tc, xt_d, x16_d, x8_d, wphiT_d, wthetaT_d, wg_d, wmT8_d, out_d)
    nc.compile()
    return nc


def _body(tc, xt_d, x16_d, x8_d, wphiT_d, wthetaT_d, wg_d, wmT8_d, out_d):
    nc = tc.nc
    from contextlib import ExitStack

    with ExitStack() as ctx:
        const = ctx.enter_context(tc.tile_pool(name="const", bufs=1))
        xtp = ctx.enter_context(tc.tile_pool(name="xtp", bufs=1))
        xin = ctx.enter_context(tc.tile_pool(name="xin", bufs=1))
        wpool = ctx.enter_context(tc.tile_pool(name="wpool", bufs=1))
        mid = ctx.enter_context(tc.tile_pool(name="mid", bufs=1))
        vecs = ctx.enter_context(tc.tile_pool(name="vecs", bufs=8))
        outp = ctx.enter_context(tc.tile_pool(name="outp", bufs=4))
        psA = ctx.enter_context(tc.tile_pool(name="psA", bufs=2, space="PSUM"))
        psB = ctx.enter_context(tc.tile_pool(name="psB", bufs=2, space="PSUM"))

        identity = const.tile([P, P], FP32)
        make_identity(nc, identity)

        # ~12 throwaway matmuls warm the PE (p-state ramps to full clock
        # after ~3us of activity) while the first xt chunk is in flight.
        warm = psB.tile([P, 2, C], FP32, tag="psB")
        for _ in range(12):
            nc.tensor.matmul(
                warm[:, 0, :P], identity, identity, start=True, stop=True
            )

        # ---- input DMA schedule.
        # sync HWDGE:   xt even chunks, then x16 left half; stores later.
        # scalar HWDGE: xt odd chunks, then w_theta, w_phi, wg, x16 right half.
        # gpsimd SWDGE: x8, wmT8 (needed latest, software queue).
        xt_sb = xtp.tile([P, NT, C], FP16)
        x16_sb = xin.tile([P, CC, N], FP16)
        x8_sb = xin.tile([P, CC, N], FP8)
        wphiT = wpool.tile([P, CC, C], FP32R)
        wthetaT = wpool.tile([P, CC, C], FP32R)
        wg16 = wpool.tile([P, CC, C], FP16)
        wmT8 = wpool.tile([P, CC, C], FP8)

        NCHUNK = 16
        TPC = NT // NCHUNK  # 2 tiles per chunk
        for k in range(NCHUNK):
            eng = nc.sync if (k % 2 == 0) else nc.scalar
            eng.dma_start(
                out=xt_sb[:, k * TPC:(k + 1) * TPC, :],
                in_=xt_d[k * TPC * P:(k + 1) * TPC * P, :].rearrange(
                    "(tt p) c -> p tt c", p=P
                ),
            )
        nc.scalar.dma_start(
            out=wthetaT, in_=wthetaT_d.rearrange("(cc p) j -> p cc j", p=P)
        )
        nc.scalar.dma_start(
            out=wphiT, in_=wphiT_d.rearrange("(cc p) j -> p cc j", p=P)
        )
        nc.scalar.dma_start(
            out=wg16, in_=wg_d.rearrange("(cc p) j -> p cc j", p=P)
        )
        nc.gpsimd.dma_start(
            out=x8_sb, in_=x8_d.rearrange("(cc p) n -> p cc n", p=P)
        )
        nc.gpsimd.dma_start(
            out=wmT8, in_=wmT8_d.rearrange("(cc p) j -> p cc j", p=P)
        )
        HN = N // 2
        nc.sync.dma_start(
            out=x16_sb[:, :, :HN],
            in_=x16_d[:, :HN].rearrange("(cc p) n -> p cc n", p=P),
        )
        nc.scalar.dma_start(
            out=x16_sb[:, :, HN:],
            in_=x16_d[:, HN:].rearrange("(cc p) n -> p cc n", p=P),
        )

        # ---- Gram, upper triangle only: G[mc-block, 128*mc:] accumulated
        # over the 32 streamed xt tiles. gaccA = rows 0,1; gaccB = rows 2,3.
        gaccA = psA.tile([P, 2, C], FP32, tag="psA")
        gaccB = psA.tile([P, 2, C], FP32, tag="psA")
        gacc = [
            (gaccA[:, 0, :], 0), (gaccA[:, 1, :C - P], P),
            (gaccB[:, 0, :C - 2 * P], 2 * P), (gaccB[:, 1, :C - 3 * P], 3 * P),
        ]
        for k in range(NCHUNK):
            for tt in range(TPC):
                t = k * TPC + tt
                for mc in range(CC):
                    dst, col0 = gacc[mc]
                    nc.tensor.matmul(
                        dst,
                        xt_sb[:, t, mc * P:(mc + 1) * P],
                        xt_sb[:, t, col0:],
                        start=(t == 0),
                        stop=(t == NT - 1),
                    )

        # full G materialized in SBUF: triangle rows + 6 transposed blocks
        g_sb = mid.tile([P, CC, C], FP32R)
        for mc in range(CC):
            src, col0 = gacc[mc]
            nc.any.tensor_copy(g_sb[:, mc, col0:], src)
        # lower blocks (a > b): G[a, b-block] = T(G[b, a-block])
        for a in range(CC):
            for b in range(a):
                pt = psB.tile([P, 2, C], FP32, tag="psB")
                nc.tensor.transpose(
                    pt[:, 0, :P],
                    _f32(g_sb[:, b, a * P:(a + 1) * P]),
                    identity,
                )
                nc.any.tensor_copy(
                    g_sb[:, a, b * P:(b + 1) * P], pt[:, 0, :P]
                )

        # ---- T1 = G @ Wt^T  (G blocks stationary)
        t1_sb = mid.tile([P, CC, C], FP32R)
        for half in range(2):
            tp = psB.tile([P, 2, C], FP32, tag="psB")
            for sub in range(2):
                mc = half * 2 + sub
                for jc in range(CC):
                    nc.tensor.matmul(
                        tp[:, sub, :],
                        g_sb[:, jc, mc * P:(mc + 1) * P],
                        wthetaT[:, jc, :],
                        start=(jc == 0),
                        stop=(jc == CC - 1),
                    )
                nc.any.tensor_copy(t1_sb[:, mc, :], tp[:, sub, :])

        # ---- L = Wp @ T1 ; softmax rows -> attn (fp32)
        attn_sb = mid.tile([P, CC, C], FP32)
        for half in range(2):
            lp = psB.tile([P, 2, C], FP32, tag="psB")
            for sub in range(2):
                mc = half * 2 + sub
                for ic in range(CC):
                    nc.tensor.matmul(
                        lp[:, sub, :],
                        wphiT[:, ic, mc * P:(mc + 1) * P],
                        t1_sb[:, ic, :],
                        start=(ic == 0),
                        stop=(ic == CC - 1),
                    )
                neg_max = vecs.tile([P, 1], FP32)
                nc.vector.tensor_reduce(
                    out=neg_max, in_=lp[:, sub, :], axis=mybir.AxisListType.X,
                    op=mybir.AluOpType.max, negate=True,
                )
                sums = vecs.tile([P, 1], FP32)
                nc.scalar.activation(
                    out=attn_sb[:, mc, :], in_=lp[:, sub, :],
                    func=mybir.ActivationFunctionType.Exp,
                    bias=neg_max, scale=1.0, accum_out=sums,
                )
                rinv = vecs.tile([P, 1], FP32)
                nc.vector.reciprocal(rinv, sums)
                nc.vector.tensor_scalar_mul(
                    attn_sb[:, mc, :], attn_sb[:, mc, :], rinv
                )

        # ---- attn^T via PE transposes, cast to fp16 on the PSUM->SBUF copy
        attnT16 = mid.tile([P, CC, C], FP16)
        for half in range(2):
            pt = psB.tile([P, 2, C], FP32, tag="psB")
            for sub in range(2):
                dc = half * 2 + sub
                for mc in range(CC):
                    nc.tensor.transpose(
                        pt[:, sub, mc * P:(mc + 1) * P],
                        attn_sb[:, mc, dc * P:(dc + 1) * P],
                        identity,
                    )
                nc.any.tensor_copy(attnT16[:, dc, :], pt[:, sub, :])

        # ---- A'^T[j, c] = sum_d Wg[d, j] attn[c, d]; cast to fp8 * S_A
        apT8 = mid.tile([P, CC, C], FP8)
        for half in range(2):
            ap_ps = psB.tile([P, 2, C], FP32, tag="psB")
            for sub in range(2):
                jc = half * 2 + sub
                for dc in range(CC):
                    nc.tensor.matmul(
                        ap_ps[:, sub, :],
                        wg16[:, dc, jc * P:(jc + 1) * P],
                        attnT16[:, dc, :],
                        start=(dc == 0),
                        stop=(dc == CC - 1),
                    )
                nc.scalar.activation(
                    out=apT8[:, jc, :], in_=ap_ps[:, sub, :],
                    func=mybir.ActivationFunctionType.Copy, scale=S_A,
                )

        # ---- ZS (y^T blocks in Z layout) + mask GEMM + residual + store,
        # both in fp8 DoubleRow (K=256 per pass). Software-pipelined:
        # mask(q-1) runs on PE while ZS(q)'s PSUM->fp8 casts run on ACT/DVE.
        zs8 = mid.tile([P, CC, N], FP8)
        xr8 = x8_sb.rearrange("p cc (ci m q) -> p cc ci q m", ci=CC, q=QF)

        def zs_pass(q):
            for ci2 in range(2):
                zp = psA.tile([P, 2, C], FP32, tag="psA")
                for s in range(2):
                    ci = ci2 * 2 + s
                    for j2 in range(2):
                        nc.tensor.matmul(
                            zp[:, s, :],
                            xr8[:, 2 * j2:2 * j2 + 2, ci, q, :],
                            apT8[:, 2 * j2:2 * j2 + 2, :],
                            start=(j2 == 0),
                            stop=(j2 == 1),
                            perf_mode=mybir.MatmulPerfMode.DoubleRow,
                        )
                eng = nc.scalar if (q + ci2) % 2 == 0 else nc.vector
                if eng is nc.scalar:
                    nc.scalar.activation(
                        out=zs8[:, 2 * ci2:2 * ci2 + 2, q * C:(q + 1) * C],
                        in_=zp, func=mybir.ActivationFunctionType.Copy,
                        scale=ZCAST,
                    )
                else:
                    nc.vector.tensor_scalar_mul(
                        zs8[:, 2 * ci2:2 * ci2 + 2, q * C:(q + 1) * C],
                        zp, ZCAST,
                    )

        def mask_pass(q):
            for oc2 in range(2):
                mp = psB.tile([P, 2, C], FP32, tag="psB")
                # residual prefill: PSUM = x * S_MX, matmuls accumulate on top
                peng = nc.vector if (q + oc2) % 2 == 0 else nc.scalar
                if peng is nc.vector:
                    nc.vector.tensor_scalar_mul(
                        mp, x16_sb[:, 2 * oc2:2 * oc2 + 2, q * C:(q + 1) * C],
                        S_MX,
                    )
                else:
                    nc.scalar.activation(
                        out=mp,
                        in_=x16_sb[:, 2 * oc2:2 * oc2 + 2, q * C:(q + 1) * C],
                        func=mybir.ActivationFunctionType.Copy, scale=S_MX,
                    )
                for s in range(2):
                    oc = oc2 * 2 + s
                    for i2 in range(2):
                        nc.tensor.matmul(
                            mp[:, s, :],
                            wmT8[:, 2 * i2:2 * i2 + 2, oc * P:(oc + 1) * P],
                            zs8[:, 2 * i2:2 * i2 + 2, q * C:(q + 1) * C],
                            start=False,
                            stop=(i2 == 1),
                            perf_mode=mybir.MatmulPerfMode.DoubleRow,
                            skip_group_check=True,
                        )
                ot = outp.tile([P, 2, C], FP16)
                feng = nc.scalar if (q + oc2) % 2 == 0 else nc.vector
                if feng is nc.scalar:
                    nc.scalar.activation(
                        out=ot, in_=mp,
                        func=mybir.ActivationFunctionType.Copy, scale=1.0 / S_MX,
                    )
                else:
                    nc.vector.tensor_scalar_mul(ot, mp, 1.0 / S_MX)
                nc.sync.dma_start(
                    out=out_d[
                        oc2 * 2 * P:(oc2 * 2 + 2) * P, q * C:(q + 1) * C
                    ].rearrange("(cc p) n -> p cc n", p=P),
                    in_=ot,
                )

        for q in range(QF):
            zs_pass(q)
            if q > 0:
                mask_pass(q - 1)
        mask_pass(QF - 1)


_NC_CACHE = {}
LAST_RESULT = None


def get_nc():
    if "nc" not in _NC_CACHE:
        _NC_CACHE["nc"] = _build_nc()
    return _NC_CACHE["nc"]


def _round_fp32r(x):
    """Round fp32 array to the fp32r grid (11 explicit mantissa bits, RNE)."""
    u = np.ascontiguousarray(x, dtype=np.float32).view(np.uint32).astype(np.uint64)
    shift = 23 - 11
    add = (np.uint64(1) << np.uint64(shift - 1)) - np.uint64(1) + (
        (u >> np.uint64(shift)) & np.uint64(1)
    )
    u = (u + add) & np.uint64(~((1 << shift) - 1) & 0xFFFFFFFF)
    return u.astype(np.uint32).view(np.float32)


def _e4m3(a):
    return np.asarray(
        np.clip(np.asarray(a, np.float32), -448.0, 448.0),
        ml_dtypes.float8_e4m3fn,
    )


def make_in_map(xb, w_phi_t, w_theta_t, w_g16, w_mask_t8):
    """Per-core input dict; xb is one sample [C, H, W]."""
    xf = np.ascontiguousarray(xb.reshape(C, N), dtype=np.float32)
    return {
        "xt16": np.ascontiguousarray(xf.T).astype(np.float16),
        "x16": xf.astype(np.float16),
        "x8": _e4m3(xf * S_X),
        "w_phi_t": w_phi_t,
        "w_theta_t": w_theta_t,
        "w_g16": w_g16,
        "w_mask_t8": w_mask_t8,
    }


def prep_weights(w_phi, w_theta, w_g, w_mask, gamma):
    w_phi_t = _round_fp32r(np.asarray(w_phi, dtype=np.float32).T)
    w_theta_t = _round_fp32r(np.asarray(w_theta, dtype=np.float32).T)
    w_g16 = np.asarray(w_g, dtype=np.float32).astype(np.float16)
    gamma64 = float(np.asarray(gamma, dtype=np.float32).reshape(-1)[0])
    w_mask_t8 = _e4m3(
        (np.asarray(w_mask, dtype=np.float64).T * gamma64 * S_M).astype(np.float32)
    )
    return w_phi_t, w_theta_t, w_g16, w_mask_t8


def kernel(x, w_phi, w_theta, w_g, w_mask, gamma):
    global LAST_RESULT
    x = np.ascontiguousarray(np.asarray(x, dtype=np.float32))
    B, c, h, w = x.shape
    assert (c, h * w) == (C, N), (x.shape,)

    w_phi_t, w_theta_t, w_g16, w_mask_t8 = prep_weights(
        w_phi, w_theta, w_g, w_mask, gamma
    )
    nc = get_nc()
    in_maps = [
        make_in_map(x[b], w_phi_t, w_theta_t, w_g16, w_mask_t8)
        for b in range(B)
    ]
    trace = bool(int(os.environ.get("KERNEL_TRACE", "0")))
    res = run_bass_kernel_spmd(nc, in_maps, list(range(B)), trace=trace)
    LAST_RESULT = res
    out = np.stack([
        np.asarray(res.results[b]["out"], dtype=np.float32).reshape(c, h, w)
        for b in range(B)
    ])
    return out


# revision 5
# speedup vs baseline: 1.5095x; 1.0448x over previous
# Trainium2 Bass kernel for per-sample channel-attention module (CAM).
#
# Reference math per sample (C=512, N=H*W=4096):
#   X = x.reshape(C, N)
#   phi = Wp X ; theta = Wt X ; g = Wg X
#   attn = softmax_rows(phi @ theta^T)          # [C, C]
#   y = attn @ g                                 # [C, N]
#   Z = (y^T).flatten().reshape(C, N)            # torch permute+view reinterpretation
#   out = gamma * (Wm @ Z) + x
#
# Algebraic restructuring (cuts PE work ~1.8x vs the naive 6-GEMM chain):
#   G = X X^T                  (Gram, [C, C])
#   L = Wp G Wt^T              (attention logits via two small GEMMs)
#   A' = softmax(L) @ Wg       (fold g-projection into attention)
#   y = A' X                   (single big GEMM)
# The torch permute+view reinterpretation is free: y^T blocks are produced
# with a stride-8 column selection of X as the stationary matmul operand, so
# each PSUM tile lands exactly on a contiguous block of Z's SBUF layout.
#
# Mixed precision (validated against the fp64 reference; the softmax here is
# a hard argmax with large top1-top2 logit gaps, so post-softmax stages are
# linear in quantization error while the logit path needs >=10 bits):
#   - logit path (X^T stream, Gram, G, T1, Wp, Wt): fp16 — fp16 weights get
#     the automatic fast-weight-load path so LDWEIGHTS hides behind matmuls
#   - attn: fp16 (fp32 PSUM + exact max-subtraction in the softmax)
#   - A', X, Z, gamma*Wm^T: fp8 e4m3 with power-of-2 scales; ZS and mask
#     GEMMs run in DoubleRow perf mode (K=256 per pass, ~2x fp16 rate)
#   - residual: fp16 x, added via PSUM prefill (scale 2^16) so the final
#     PSUM->SBUF copy is a single scaled cast to the fp16 output
# Gram exploits symmetry: only upper-triangle blocks are computed (1280 of
# 2048 moving columns per tile); the 6 lower blocks come from PE transposes.

import os
import numpy as np
import ml_dtypes

import concourse.bass as bass
import concourse.mybir as mybir
import concourse.tile as tile
from concourse import bacc
from concourse.bass_utils import run_bass_kernel_spmd
from concourse.tile import TileContext

P = 128          # partitions
C = 512          # channels
N = 4096         # spatial (64*64)
CC = C // P      # 4 channel chunks
NT = N // P      # 32 spatial tiles
QF = N // C      # 8 fold factor for the permute+view reinterpretation
FP32 = mybir.dt.float32
FP16 = mybir.dt.float16
FP8 = mybir.dt.float8e4

S_X = 8.0        # x fp8 scale
S_A = 512.0      # A' fp8 scale
S_M = 8192.0     # gamma*Wm^T fp8 scale
S_MX = S_M * S_X           # 65536: residual prefill scale
ZCAST = S_X / (S_A * S_X)  # PSUM (y*S_A*S_X) -> Z*S_X


def _build_nc():
    nc = bacc.Bacc("TRN2", target_bir_lowering=False, debug=False, num_devices=8)
    id_d = nc.dram_tensor("id16", [P, P], FP16, kind="ExternalInput").ap()
    xt_d = nc.dram_tensor("xt16", [N, C], FP16, kind="ExternalInput").ap()
    x16_d = nc.dram_tensor("x16", [C, N], FP16, kind="ExternalInput").ap()
    x8_d = nc.dram_tensor("x8", [C, N], FP8, kind="ExternalInput").ap()
    wphiT_d = nc.dram_tensor("w_phi_t16", [C, C], FP16, kind="ExternalInput").ap()
    wthetaT_d = nc.dram_tensor("w_theta_t16", [C, C], FP16, kind="ExternalInput").ap()
    wg_d = nc.dram_tensor("w_g16", [C, C], FP16, kind="ExternalInput").ap()
    wmT8_d = nc.dram_tensor("w_mask_t8", [C, C], FP8, kind="ExternalInput").ap()
    out_d = nc.dram_tensor("out", [C, N], FP16, kind="ExternalOutput").ap()

    with TileContext(nc) as tc:
        _body(tc, id_d, xt_d, x16_d, x8_d, wphiT_d, wthetaT_d, wg_d, wmT8_d, out_d)
    nc.compile()
    return nc


def _body(tc, id_d, xt_d, x16_d, x8_d, wphiT_d, wthetaT_d, wg_d, wmT8_d, out_d):
    nc = tc.nc
    from contextlib import ExitStack

    with ExitStack() as ctx:
        const = ctx.enter_context(tc.tile_pool(name="const", bufs=1))
        xtp = ctx.enter_context(tc.tile_pool(name="xtp", bufs=1))
        xin = ctx.enter_context(tc.tile_pool(name="xin", bufs=1))
        wpool = ctx.enter_context(tc.tile_pool(name="wpool", bufs=1))
        mid = ctx.enter_context(tc.tile_pool(name="mid", bufs=1))
        vecs = ctx.enter_context(tc.tile_pool(name="vecs", bufs=8))
        outp = ctx.enter_context(tc.tile_pool(name="outp", bufs=4))
        psA = ctx.enter_context(tc.tile_pool(name="psA", bufs=2, space="PSUM"))
        psB = ctx.enter_context(tc.tile_pool(name="psB", bufs=2, space="PSUM"))

        # Warm source: DVE memset (no gpsimd in the startup path). ~12
        # throwaway matmuls ramp the PE p-state while DMA streams in.
        wsrc = const.tile([P, C], FP16)
        nc.vector.memset(wsrc, 1.0)
        warm = psB.tile([P, 2, C], FP32, tag="psB")
        for _ in range(12):
            nc.tensor.matmul(
                warm[:, 0, :], wsrc[:, :P], wsrc, start=True, stop=True
            )

        # ---- input DMA schedule.
        # sync HWDGE:   xt even chunks, x16 left half; output stores later.
        # scalar HWDGE: identity, xt odd chunks, w_theta, w_phi, wg, x16 right.
        # gpsimd SWDGE: x8, wmT8 (needed latest; software queue).
        id16 = const.tile([P, P], FP16)
        xt_sb = xtp.tile([P, NT, C], FP16)
        x16_sb = xin.tile([P, CC, N], FP16)
        x8_sb = xin.tile([P, CC, N], FP8)
        wphiT = wpool.tile([P, CC, C], FP16)
        wthetaT = wpool.tile([P, CC, C], FP16)
        wg16 = wpool.tile([P, CC, C], FP16)
        wmT8 = wpool.tile([P, CC, C], FP8)

        nc.scalar.dma_start(out=id16, in_=id_d)
        NCHUNK = 16
        TPC = NT // NCHUNK  # 2 tiles per chunk
        for k in range(NCHUNK):
            eng = nc.sync if (k % 2 == 0) else nc.scalar
            eng.dma_start(
                out=xt_sb[:, k * TPC:(k + 1) * TPC, :],
                in_=xt_d[k * TPC * P:(k + 1) * TPC * P, :].rearrange(
                    "(tt p) c -> p tt c", p=P
                ),
            )
        nc.scalar.dma_start(
            out=wthetaT, in_=wthetaT_d.rearrange("(cc p) j -> p cc j", p=P)
        )
        nc.scalar.dma_start(
            out=wphiT, in_=wphiT_d.rearrange("(cc p) j -> p cc j", p=P)
        )
        nc.scalar.dma_start(
            out=wg16, in_=wg_d.rearrange("(cc p) j -> p cc j", p=P)
        )
        nc.gpsimd.dma_start(
            out=x8_sb, in_=x8_d.rearrange("(cc p) n -> p cc n", p=P)
        )
        nc.gpsimd.dma_start(
            out=wmT8, in_=wmT8_d.rearrange("(cc p) j -> p cc j", p=P)
        )
        HN = N // 2
        nc.sync.dma_start(
            out=x16_sb[:, :, :HN],
            in_=x16_d[:, :HN].rearrange("(cc p) n -> p cc n", p=P),
        )
        nc.scalar.dma_start(
            out=x16_sb[:, :, HN:],
            in_=x16_d[:, HN:].rearrange("(cc p) n -> p cc n", p=P),
        )

        # ---- Gram, upper triangle only: G[mc-block, 128*mc:] accumulated
        # over the 32 streamed xt tiles. gaccA = rows 0,1; gaccB = rows 2,3.
        gaccA = psA.tile([P, 2, C], FP32, tag="psA")
        gaccB = psA.tile([P, 2, C], FP32, tag="psA")
        gacc = [
            (gaccA[:, 0, :], 0), (gaccA[:, 1, :C - P], P),
            (gaccB[:, 0, :C - 2 * P], 2 * P), (gaccB[:, 1, :C - 3 * P], 3 * P),
        ]
        for k in range(NCHUNK):
            for tt in range(TPC):
                t = k * TPC + tt
                for mc in range(CC):
                    dst, col0 = gacc[mc]
                    nc.tensor.matmul(
                        dst,
                        xt_sb[:, t, mc * P:(mc + 1) * P],
                        xt_sb[:, t, col0:],
                        start=(t == 0),
                        stop=(t == NT - 1),
                    )

        # full G (fp16) in SBUF: triangle rows + 6 transposed lower blocks
        g_sb = mid.tile([P, CC, C], FP16)
        for mc in range(CC):
            src, col0 = gacc[mc]
            if mc % 2 == 0:
                nc.scalar.activation(
                    out=g_sb[:, mc, col0:], in_=src,
                    func=mybir.ActivationFunctionType.Copy, scale=1.0,
                )
            else:
                nc.vector.tensor_copy(g_sb[:, mc, col0:], src)

        t1_sb = mid.tile([P, CC, C], FP16)
        tps = {}
        tA = psB.tile([P, 2, C], FP32, tag="psB")
        tps[3], tps[2] = tA[:, 0, :], tA[:, 1, :]
        tB = psB.tile([P, 2, C], FP32, tag="psB")
        tps[1], tps[0] = tB[:, 0, :], tB[:, 1, :]

        def t1_pass(mc):
            # T1 = G @ Wt^T (G blocks stationary). mc=3 uses only triangle
            # rows; other mc need the transposed lower blocks.
            tp = tps[mc]
            for jc in range(CC):
                nc.tensor.matmul(
                    tp,
                    g_sb[:, jc, mc * P:(mc + 1) * P],
                    wthetaT[:, jc, :],
                    start=(jc == 0),
                    stop=(jc == CC - 1),
                )
            nc.any.tensor_copy(t1_sb[:, mc, :], tp)

        t1_pass(3)
        # lower blocks (a > b): G[a, b-block] = T(G[b, a-block]); packed into
        # one fp16 PSUM tile, then copied back into g_sb.
        gt = psA.tile([P, 2, 4, P], FP16, tag="psA")
        lower = [(1, 0), (2, 0), (2, 1), (3, 0), (3, 1), (3, 2)]
        for i, (a, b) in enumerate(lower):
            nc.tensor.transpose(
                gt[:, i // 4, i % 4, :],
                g_sb[:, b, a * P:(a + 1) * P],
                id16,
            )
        for i, (a, b) in enumerate(lower):
            nc.any.tensor_copy(
                g_sb[:, a, b * P:(b + 1) * P], gt[:, i // 4, i % 4, :]
            )
        t1_pass(2)
        t1_pass(1)
        t1_pass(0)

        # ---- L = Wp @ T1 ; softmax rows -> attn (fp16); attn^T transposes
        # interleaved per-mc so the PE never waits on a softmax.
        attn_sb = mid.tile([P, CC, C], FP16)
        attnT16 = mid.tile([P, CC, C], FP16)
        ptA = psA.tile([P, 2, C], FP16, tag="psA")  # attnT rows dc=0,1
        ptB = psA.tile([P, 2, C], FP16, tag="psA")  # attnT rows dc=2,3
        pt = {0: ptA[:, 0, :], 1: ptA[:, 1, :], 2: ptB[:, 0, :], 3: ptB[:, 1, :]}
        lps = {}

        def l_pass(mc):
            if mc % 2 == 0:
                lp2 = psB.tile([P, 2, C], FP32, tag="psB")
                lps[mc], lps[mc + 1] = lp2[:, 0, :], lp2[:, 1, :]
            lp = lps[mc]
            for ic in range(CC):
                nc.tensor.matmul(
                    lp,
                    wphiT[:, ic, mc * P:(mc + 1) * P],
                    t1_sb[:, ic, :],
                    start=(ic == 0),
                    stop=(ic == CC - 1),
                )
            neg_max = vecs.tile([P, 1], FP32)
            nc.vector.tensor_reduce(
                out=neg_max, in_=lp, axis=mybir.AxisListType.X,
                op=mybir.AluOpType.max, negate=True,
            )
            sums = vecs.tile([P, 1], FP32)
            nc.scalar.activation(
                out=attn_sb[:, mc, :], in_=lp,
                func=mybir.ActivationFunctionType.Exp,
                bias=neg_max, scale=1.0, accum_out=sums,
            )
            rinv = vecs.tile([P, 1], FP32)
            nc.vector.reciprocal(rinv, sums)
            nc.vector.tensor_scalar_mul(
                attn_sb[:, mc, :], attn_sb[:, mc, :], rinv
            )

        def at_pass(mc):
            for dc in range(CC):
                nc.tensor.transpose(
                    pt[dc][:, mc * P:(mc + 1) * P],
                    attn_sb[:, mc, dc * P:(dc + 1) * P],
                    id16,
                )

        l_pass(0)
        l_pass(1)
        at_pass(0)
        l_pass(2)
        at_pass(1)
        l_pass(3)
        at_pass(2)
        at_pass(3)
        for dc in range(CC):
            nc.any.tensor_copy(attnT16[:, dc, :], pt[dc])

        # ---- A'^T[j, c] = sum_d Wg[d, j] attn[c, d]; cast to fp8 * S_A
        apT8 = mid.tile([P, CC, C], FP8)
        for half in range(2):
            ap_ps = psB.tile([P, 2, C], FP32, tag="psB")
            for sub in range(2):
                jc = half * 2 + sub
                for dc in range(CC):
                    nc.tensor.matmul(
                        ap_ps[:, sub, :],
                        wg16[:, dc, jc * P:(jc + 1) * P],
                        attnT16[:, dc, :],
                        start=(dc == 0),
                        stop=(dc == CC - 1),
                    )
                nc.scalar.activation(
                    out=apT8[:, jc, :], in_=ap_ps[:, sub, :],
                    func=mybir.ActivationFunctionType.Copy, scale=S_A,
                )

        # ---- ZS (y^T blocks in Z layout) + mask GEMM + residual + store,
        # both in fp8 DoubleRow (K=256 per pass). Software-pipelined:
        # mask(q-1) runs on PE while ZS(q)'s PSUM->fp8 casts run on ACT/DVE.
        zs8 = mid.tile([P, CC, N], FP8)
        xr8 = x8_sb.rearrange("p cc (ci m q) -> p cc ci q m", ci=CC, q=QF)

        def zs_pass(q):
            for ci2 in range(2):
                zp = psA.tile([P, 2, C], FP32, tag="psA")
                for s in range(2):
                    ci = ci2 * 2 + s
                    for j2 in range(2):
                        nc.tensor.matmul(
                            zp[:, s, :],
                            xr8[:, 2 * j2:2 * j2 + 2, ci, q, :],
                            apT8[:, 2 * j2:2 * j2 + 2, :],
                            start=(j2 == 0),
                            stop=(j2 == 1),
                            perf_mode=mybir.MatmulPerfMode.DoubleRow,
                        )
                if (q + ci2) % 2 == 0:
                    nc.scalar.activation(
                        out=zs8[:, 2 * ci2:2 * ci2 + 2, q * C:(q + 1) * C],
                        in_=zp, func=mybir.ActivationFunctionType.Copy,
                        scale=ZCAST,
                    )
                else:
                    nc.vector.tensor_scalar_mul(
                        zs8[:, 2 * ci2:2 * ci2 + 2, q * C:(q + 1) * C],
                        zp, ZCAST,
                    )

        def mask_pass(q):
            for oc2 in range(2):
                mp = psB.tile([P, 2, C], FP32, tag="psB")
                # residual prefill: PSUM = x * S_MX, matmuls accumulate on top
                if (q + oc2) % 2 == 0:
                    nc.vector.tensor_scalar_mul(
                        mp, x16_sb[:, 2 * oc2:2 * oc2 + 2, q * C:(q + 1) * C],
                        S_MX,
                    )
                else:
                    nc.scalar.activation(
                        out=mp,
                        in_=x16_sb[:, 2 * oc2:2 * oc2 + 2, q * C:(q + 1) * C],
                        func=mybir.ActivationFunctionType.Copy, scale=S_MX,
                    )
                for s in range(2):
                    oc = oc2 * 2 + s
                    for i2 in range(2):
                        nc.tensor.matmul(
                            mp[:, s, :],
                            wmT8[:, 2 * i2:2 * i2 + 2, oc * P:(oc + 1) * P],
                            zs8[:, 2 * i2:2 * i2 + 2, q * C:(q + 1) * C],
                            start=False,
                            stop=(i2 == 1),
                            perf_mode=mybir.MatmulPerfMode.DoubleRow,
                            skip_group_check=True,
                        )
                ot = outp.tile([P, 2, C], FP16)
                if (q + oc2) % 2 == 0:
                    nc.scalar.activation(
                        out=ot, in_=mp,
                        func=mybir.ActivationFunctionType.Copy, scale=1.0 / S_MX,
                    )
                else:
                    nc.vector.tensor_scalar_mul(ot, mp, 1.0 / S_MX)
                nc.sync.dma_start(
                    out=out_d[
                        oc2 * 2 * P:(oc2 * 2 + 2) * P, q * C:(q + 1) * C
                    ].rearrange("(cc p) n -> p cc n", p=P),
                    in_=ot,
                )

        for q in range(QF):
            zs_pass(q)
            if q > 0:
                mask_pass(q - 1)
        mask_pass(QF - 1)


_NC_CACHE = {}
LAST_RESULT = None


def get_nc():
    if "nc" not in _NC_CACHE:
        _NC_CACHE["nc"] = _build_nc()
    return _NC_CACHE["nc"]


def _e4m3(a):
    return np.asarray(
        np.clip(np.asarray(a, np.float32), -448.0, 448.0),
        ml_dtypes.float8_e4m3fn,
    )


_ID16 = np.eye(P, dtype=np.float16)


def make_in_map(xb, w_phi_t16, w_theta_t16, w_g16, w_mask_t8):
    """Per-core input dict; xb is one sample [C, H, W]."""
    xf = np.ascontiguousarray(xb.reshape(C, N), dtype=np.float32)
    return {
        "id16": _ID16,
        "xt16": np.ascontiguousarray(xf.T).astype(np.float16),
        "x16": xf.astype(np.float16),
        "x8": _e4m3(xf * S_X),
        "w_phi_t16": w_phi_t16,
        "w_theta_t16": w_theta_t16,
        "w_g16": w_g16,
        "w_mask_t8": w_mask_t8,
    }


def prep_weights(w_phi, w_theta, w_g, w_mask, gamma):
    w_phi_t16 = np.asarray(w_phi, dtype=np.float32).T.astype(np.float16)
    w_theta_t16 = np.asarray(w_theta, dtype=np.float32).T.astype(np.float16)
    w_g16 = np.asarray(w_g, dtype=np.float32).astype(np.float16)
    gamma64 = float(np.asarray(gamma, dtype=np.float32).reshape(-1)[0])
    w_mask_t8 = _e4m3(
        (np.asarray(w_mask, dtype=np.float64).T * gamma64 * S_M).astype(np.float32)
    )
    return w_phi_t16, w_theta_t16, w_g16, w_mask_t8


def kernel(x, w_phi, w_theta, w_g, w_mask, gamma):
    global LAST_RESULT
    x = np.ascontiguousarray(np.asarray(x, dtype=np.float32))
    B, c, h, w = x.shape
    assert (c, h * w) == (C, N), (x.shape,)

    w_phi_t16, w_theta_t16, w_g16, w_mask_t8 = prep_weights(
        w_phi, w_theta, w_g, w_mask, gamma
    )
    nc = get_nc()
    in_maps = [
        make_in_map(x[b], w_phi_t16, w_theta_t16, w_g16, w_mask_t8)
        for b in range(B)
    ]
    trace = bool(int(os.environ.get("KERNEL_TRACE", "0")))
    res = run_bass_kernel_spmd(nc, in_maps, list(range(B)), trace=trace)
    LAST_RESULT = res
    out = np.stack([
        np.asarray(res.results[b]["out"], dtype=np.float32).reshape(c, h, w)
        for b in range(B)
    ])
    return out


# revision 10
# speedup vs baseline: 1.5924x; 1.0549x over previous
# Trainium2 Bass kernel for per-sample channel-attention module (CAM).
#
# Reference math per sample (C=512, N=H*W=4096):
#   X = x.reshape(C, N)
#   phi = Wp X ; theta = Wt X ; g = Wg X
#   attn = softmax_rows(phi @ theta^T)          # [C, C]
#   y = attn @ g                                 # [C, N]
#   Z = (y^T).flatten().reshape(C, N)            # torch permute+view reinterpretation
#   out = gamma * (Wm @ Z) + x
#
# Algebraic restructuring (cuts PE work ~1.8x vs the naive 6-GEMM chain):
#   G = X X^T                  (Gram, [C, C])
#   L = Wp G Wt^T              (attention logits via two small GEMMs)
#   A' = softmax(L) @ Wg       (fold g-projection into attention)
#   y = A' X                   (single big GEMM)
# The torch permute+view reinterpretation is free: y^T blocks are produced
# with a stride-8 column selection of X as the stationary matmul operand, so
# each PSUM tile lands exactly on a contiguous block of Z's SBUF layout.
#
# Mixed precision (validated against the fp64 reference; the softmax here is
# a hard argmax with large top1-top2 logit gaps, so post-softmax stages are
# linear in quantization error while the logit path needs >=10 bits):
#   - logit path (X^T stream, Gram, G, T1, Wp, Wt): fp16 — fp16 weights get
#     the automatic fast-weight-load path so LDWEIGHTS hides behind matmuls
#   - attn: fp16 (fp32 PSUM + exact max-subtraction in the softmax)
#   - A', X, Z, gamma*Wm^T: fp8 e4m3 with power-of-2 scales; ZS and mask
#     GEMMs run in DoubleRow perf mode (K=256 per pass, ~2x fp16 rate)
#   - residual: fp16 x, added via PSUM prefill (scale 2^16) so the final
#     PSUM->SBUF copy is a single scaled cast to the fp16 output
# Gram exploits symmetry: only upper-triangle blocks are computed (1280 of
# 2048 moving columns per tile); the 6 lower blocks come from PE transposes.

import os
import numpy as np
import ml_dtypes

import concourse.bass as bass
import concourse.mybir as mybir
import concourse.tile as tile
from concourse import bacc
from concourse.bass_utils import run_bass_kernel_spmd
from concourse.tile import TileContext

P = 128          # partitions
C = 512          # channels
N = 4096         # spatial (64*64)
CC = C // P      # 4 channel chunks
NT = N // P      # 32 spatial tiles
QF = N // C      # 8 fold factor for the permute+view reinterpretation
FP32 = mybir.dt.float32
FP16 = mybir.dt.float16
FP8 = mybir.dt.float8e4

S_X = 8.0        # x fp8 scale
S_A = 512.0      # A' fp8 scale
S_M = 8192.0     # gamma*Wm^T fp8 scale
S_MX = S_M * S_X           # 65536: residual prefill scale
ZCAST = S_X / (S_A * S_X)  # PSUM (y*S_A*S_X) -> Z*S_X


def _build_nc():
    nc = bacc.Bacc("TRN2", target_bir_lowering=False, debug=False, num_devices=8)
    id_d = nc.dram_tensor("id16", [P, P], FP16, kind="ExternalInput").ap()
    xt_d = nc.dram_tensor("xt16", [N, C], FP16, kind="ExternalInput").ap()
    x16_d = nc.dram_tensor("x16", [C, N], FP16, kind="ExternalInput").ap()
    x8_d = nc.dram_tensor("x8", [C, N], FP8, kind="ExternalInput").ap()
    wphiT_d = nc.dram_tensor("w_phi_t16", [C, C], FP16, kind="ExternalInput").ap()
    wthetaT_d = nc.dram_tensor("w_theta_t16", [C, C], FP16, kind="ExternalInput").ap()
    wg_d = nc.dram_tensor("w_g16", [C, C], FP16, kind="ExternalInput").ap()
    wmT8_d = nc.dram_tensor("w_mask_t8", [C, C], FP8, kind="ExternalInput").ap()
    out_d = nc.dram_tensor("out", [C, N], FP16, kind="ExternalOutput").ap()

    with TileContext(nc) as tc:
        _body(tc, id_d, xt_d, x16_d, x8_d, wphiT_d, wthetaT_d, wg_d, wmT8_d, out_d)
    nc.compile()
    return nc


def _body(tc, id_d, xt_d, x16_d, x8_d, wphiT_d, wthetaT_d, wg_d, wmT8_d, out_d):
    nc = tc.nc
    from contextlib import ExitStack

    with ExitStack() as ctx:
        const = ctx.enter_context(tc.tile_pool(name="const", bufs=1))
        xtp = ctx.enter_context(tc.tile_pool(name="xtp", bufs=1))
        xin = ctx.enter_context(tc.tile_pool(name="xin", bufs=1))
        wpool = ctx.enter_context(tc.tile_pool(name="wpool", bufs=1))
        mid = ctx.enter_context(tc.tile_pool(name="mid", bufs=1))
        vecs = ctx.enter_context(tc.tile_pool(name="vecs", bufs=8))
        outp = ctx.enter_context(tc.tile_pool(name="outp", bufs=4))
        psA = ctx.enter_context(tc.tile_pool(name="psA", bufs=2, space="PSUM"))
        psB = ctx.enter_context(tc.tile_pool(name="psB", bufs=2, space="PSUM"))

        # Warm source: DVE memset (no gpsimd in the startup path). ~18
        # throwaway matmuls ramp the PE p-state while DMA streams in.
        wsrc = const.tile([P, C], FP16)
        nc.vector.memset(wsrc, 1.0)
        warm = psB.tile([P, 2, C], FP32, tag="psB")
        for _ in range(18):
            nc.tensor.matmul(
                warm[:, 0, :], wsrc[:, :P], wsrc, start=True, stop=True
            )

        # ---- input DMA schedule.
        # sync HWDGE:   xt even chunks, x16 left half; output stores later.
        # scalar HWDGE: identity, xt odd chunks, w_theta, w_phi, wg, x16 right.
        # gpsimd SWDGE: x8, wmT8 (needed latest; software queue).
        id16 = const.tile([P, P], FP16)
        xt_sb = xtp.tile([P, NT, C], FP16)
        x16_sb = xin.tile([P, CC, N], FP16)
        x8_sb = xin.tile([P, CC, N], FP8)
        wphiT = wpool.tile([P, CC, C], FP16)
        wthetaT = wpool.tile([P, CC, C], FP16)
        wg16 = wpool.tile([P, CC, C], FP16)
        wmT8 = wpool.tile([P, CC, C], FP8)

        # xt chunks: 4 single-tile starters for an early Gram start, then
        # 2-tile chunks, alternating the two HWDGE queues.
        chunk_sizes = [1, 1, 1, 1] + [2] * 14
        t0c = 0
        for k, csz in enumerate(chunk_sizes):
            eng = nc.sync if (k % 2 == 0) else nc.scalar
            eng.dma_start(
                out=xt_sb[:, t0c:t0c + csz, :],
                in_=xt_d[t0c * P:(t0c + csz) * P, :].rearrange(
                    "(tt p) c -> p tt c", p=P
                ),
            )
            t0c += csz
        assert t0c == NT
        # weights + identity on the SWDGE path; both HWDGE queues stay on
        # the latency-critical xt/x16 streams.
        nc.gpsimd.dma_start(out=id16, in_=id_d)
        nc.gpsimd.dma_start(
            out=wthetaT, in_=wthetaT_d.rearrange("(cc p) j -> p cc j", p=P)
        )
        nc.gpsimd.dma_start(
            out=wphiT, in_=wphiT_d.rearrange("(cc p) j -> p cc j", p=P)
        )
        nc.gpsimd.dma_start(
            out=wg16, in_=wg_d.rearrange("(cc p) j -> p cc j", p=P)
        )
        nc.gpsimd.dma_start(
            out=wmT8, in_=wmT8_d.rearrange("(cc p) j -> p cc j", p=P)
        )
        nc.gpsimd.dma_start(
            out=x8_sb, in_=x8_d.rearrange("(cc p) n -> p cc n", p=P)
        )
        HN = N // 2
        nc.sync.dma_start(
            out=x16_sb[:, :, :HN],
            in_=x16_d[:, :HN].rearrange("(cc p) n -> p cc n", p=P),
        )
        nc.scalar.dma_start(
            out=x16_sb[:, :, HN:],
            in_=x16_d[:, HN:].rearrange("(cc p) n -> p cc n", p=P),
        )

        # ---- Gram, upper triangle only: G[mc-block, 128*mc:] accumulated
        # over the 32 streamed xt tiles. gaccA = rows 0,1; gaccB = rows 2,3.
        gaccA = psA.tile([P, 2, C], FP32, tag="psA")
        gaccB = psA.tile([P, 2, C], FP32, tag="psA")
        gacc = [
            (gaccA[:, 0, :], 0), (gaccA[:, 1, :C - P], P),
            (gaccB[:, 0, :C - 2 * P], 2 * P), (gaccB[:, 1, :C - 3 * P], 3 * P),
        ]
        for t in range(NT):
            for mc in range(CC):
                dst, col0 = gacc[mc]
                nc.tensor.matmul(
                    dst,
                    xt_sb[:, t, mc * P:(mc + 1) * P],
                    xt_sb[:, t, col0:],
                    start=(t == 0),
                    stop=(t == NT - 1),
                )

        # full G (fp16) in SBUF: triangle rows + 6 transposed lower blocks
        g_sb = mid.tile([P, CC, C], FP16)
        for mc in range(CC):
            src, col0 = gacc[mc]
            if mc % 2 == 0:
                nc.scalar.activation(
                    out=g_sb[:, mc, col0:], in_=src,
                    func=mybir.ActivationFunctionType.Copy, scale=1.0,
                )
            else:
                nc.vector.tensor_copy(g_sb[:, mc, col0:], src)

        t1_sb = mid.tile([P, CC, C], FP16)
        tps = {}
        tA = psB.tile([P, 2, C], FP32, tag="psB")
        tps[3], tps[2] = tA[:, 0, :], tA[:, 1, :]
        tB = psB.tile([P, 2, C], FP32, tag="psB")
        tps[1], tps[0] = tB[:, 0, :], tB[:, 1, :]

        def t1_pass(mc):
            # T1 = G @ Wt^T (G blocks stationary). mc=3 uses only triangle
            # rows; other mc need the transposed lower blocks.
            tp = tps[mc]
            for jc in range(CC):
                nc.tensor.matmul(
                    tp,
                    g_sb[:, jc, mc * P:(mc + 1) * P],
                    wthetaT[:, jc, :],
                    start=(jc == 0),
                    stop=(jc == CC - 1),
                )
            nc.any.tensor_copy(t1_sb[:, mc, :], tp)

        t1_pass(3)
        # lower blocks (a > b): G[a, b-block] = T(G[b, a-block]); packed into
        # one fp16 PSUM tile, copied back right after each transpose so each
        # T1 pass unblocks as early as possible (T1(mc) needs blocks (*,mc)).
        gt = psA.tile([P, 2, 4, P], FP16, tag="psA")
        lower = [(3, 2), (2, 1), (3, 1), (1, 0), (2, 0), (3, 0)]

        def g_fill(i):
            a, b = lower[i]
            nc.tensor.transpose(
                gt[:, i // 4, i % 4, :],
                g_sb[:, b, a * P:(a + 1) * P],
                id16,
            )
            nc.any.tensor_copy(
                g_sb[:, a, b * P:(b + 1) * P], gt[:, i // 4, i % 4, :]
            )

        g_fill(0)
        t1_pass(2)
        g_fill(1)
        g_fill(2)
        t1_pass(1)
        g_fill(3)
        g_fill(4)
        g_fill(5)
        t1_pass(0)

        # ---- L = Wp @ T1 ; softmax rows -> attn (fp16); attn^T transposes
        # interleaved per-mc so the PE never waits on a softmax.
        attn_sb = mid.tile([P, CC, C], FP16)
        attnT16 = mid.tile([P, CC, C], FP16)
        ptA = psA.tile([P, 2, C], FP16, tag="psA")  # attnT rows dc=0,1
        ptB = psA.tile([P, 2, C], FP16, tag="psA")  # attnT rows dc=2,3
        pt = {0: ptA[:, 0, :], 1: ptA[:, 1, :], 2: ptB[:, 0, :], 3: ptB[:, 1, :]}
        lps = {}

        def l_pass(mc):
            if mc % 2 == 0:
                lp2 = psB.tile([P, 2, C], FP32, tag="psB")
                lps[mc], lps[mc + 1] = lp2[:, 0, :], lp2[:, 1, :]
            lp = lps[mc]
            for ic in range(CC):
                nc.tensor.matmul(
                    lp,
                    wphiT[:, ic, mc * P:(mc + 1) * P],
                    t1_sb[:, ic, :],
                    start=(ic == 0),
                    stop=(ic == CC - 1),
                )
            neg_max = vecs.tile([P, 1], FP32)
            nc.vector.tensor_reduce(
                out=neg_max, in_=lp, axis=mybir.AxisListType.X,
                op=mybir.AluOpType.max, negate=True,
            )
            sums = vecs.tile([P, 1], FP32)
            nc.scalar.activation(
                out=attn_sb[:, mc, :], in_=lp,
                func=mybir.ActivationFunctionType.Exp,
                bias=neg_max, scale=1.0, accum_out=sums,
            )
            rinv = vecs.tile([P, 1], FP32)
            nc.vector.reciprocal(rinv, sums)
            nc.vector.tensor_scalar_mul(
                attn_sb[:, mc, :], attn_sb[:, mc, :], rinv
            )

        def at_pass(mc):
            for dc in range(CC):
                nc.tensor.transpose(
                    pt[dc][:, mc * P:(mc + 1) * P],
                    attn_sb[:, mc, dc * P:(dc + 1) * P],
                    id16,
                )

        l_pass(0)
        l_pass(1)
        at_pass(0)
        l_pass(2)
        at_pass(1)
        l_pass(3)
        at_pass(2)
        at_pass(3)
        for dc in range(CC):
            nc.any.tensor_copy(attnT16[:, dc, :], pt[dc])

        # ---- A'^T[j, c] = sum_d Wg[d, j] attn[c, d]; cast to fp8 * S_A
        apT8 = mid.tile([P, CC, C], FP8)
        for half in range(2):
            ap_ps = psB.tile([P, 2, C], FP32, tag="psB")
            for sub in range(2):
                jc = half * 2 + sub
                for dc in range(CC):
                    nc.tensor.matmul(
                        ap_ps[:, sub, :],
                        wg16[:, dc, jc * P:(jc + 1) * P],
                        attnT16[:, dc, :],
                        start=(dc == 0),
                        stop=(dc == CC - 1),
                    )
                nc.scalar.activation(
                    out=apT8[:, jc, :], in_=ap_ps[:, sub, :],
                    func=mybir.ActivationFunctionType.Copy, scale=S_A,
                )

        # ---- ZS (y^T blocks in Z layout) + mask GEMM + residual + store,
        # both in fp8 DoubleRow (K=256 per pass). Software-pipelined:
        # mask(q-1) runs on PE while ZS(q)'s PSUM->fp8 casts run on ACT/DVE.
        zs8 = mid.tile([P, CC, N], FP8)
        xr8 = x8_sb.rearrange("p cc (ci m q) -> p cc ci q m", ci=CC, q=QF)

        def zs_half(q, ci2):
            # 4 DoubleRow matmuls -> one ACT cast to fp8 (Z * S_X)
            pool = psA if ci2 == 0 else psB
            zp = pool.tile([P, 2, C], FP32, tag="psA" if ci2 == 0 else "psB")
            for s in range(2):
                ci = ci2 * 2 + s
                for j2 in range(2):
                    nc.tensor.matmul(
                        zp[:, s, :],
                        xr8[:, 2 * j2:2 * j2 + 2, ci, q, :],
                        apT8[:, 2 * j2:2 * j2 + 2, :],
                        start=(j2 == 0),
                        stop=(j2 == 1),
                        perf_mode=mybir.MatmulPerfMode.DoubleRow,
                    )
            nc.scalar.activation(
                out=zs8[:, 2 * ci2:2 * ci2 + 2, q * C:(q + 1) * C],
                in_=zp, func=mybir.ActivationFunctionType.Copy,
                scale=ZCAST,
            )

        def mask_half(q, oc2):
            # 4 DoubleRow matmuls (PSUM = mask * S_MX), then one fused DVE
            # op: out = PSUM/S_MX + x (residual), straight to fp16.
            pool = psA if oc2 == 0 else psB
            mp = pool.tile([P, 2, C], FP32, tag="psA" if oc2 == 0 else "psB")
            for s in range(2):
                oc = oc2 * 2 + s
                for i2 in range(2):
                    nc.tensor.matmul(
                        mp[:, s, :],
                        wmT8[:, 2 * i2:2 * i2 + 2, oc * P:(oc + 1) * P],
                        zs8[:, 2 * i2:2 * i2 + 2, q * C:(q + 1) * C],
                        start=(i2 == 0),
                        stop=(i2 == 1),
                        perf_mode=mybir.MatmulPerfMode.DoubleRow,
                    )
            ot = outp.tile([P, 2, C], FP16)
            nc.vector.scalar_tensor_tensor(
                out=ot, in0=mp, scalar=1.0 / S_MX,
                in1=x16_sb[:, 2 * oc2:2 * oc2 + 2, q * C:(q + 1) * C],
                op0=mybir.AluOpType.mult, op1=mybir.AluOpType.add,
            )
            nc.sync.dma_start(
                out=out_d[
                    oc2 * 2 * P:(oc2 * 2 + 2) * P, q * C:(q + 1) * C
                ].rearrange("(cc p) n -> p cc n", p=P),
                in_=ot,
            )

        for q in range(QF):
            zs_half(q, 0)
            if q > 0:
                mask_half(q - 1, 0)
            zs_half(q, 1)
            if q > 0:
                mask_half(q - 1, 1)
        mask_half(QF - 1, 0)
        mask_half(QF - 1, 1)


_NC_CACHE = {}
LAST_RESULT = None


def get_nc():
    if "nc" not in _NC_CACHE:
        _NC_CACHE["nc"] = _build_nc()
    return _NC_CACHE["nc"]


def _e4m3(a):
    return np.asarray(
        np.clip(np.asarray(a, np.float32), -448.0, 448.0),
        ml_dtypes.float8_e4m3fn,
    )


_ID16 = np.eye(P, dtype=np.float16)


def make_in_map(xb, w_phi_t16, w_theta_t16, w_g16, w_mask_t8):
    """Per-core input dict; xb is one sample [C, H, W]."""
    xf = np.ascontiguousarray(xb.reshape(C, N), dtype=np.float32)
    return {
        "id16": _ID16,
        "xt16": np.ascontiguousarray(xf.T).astype(np.float16),
        "x16": xf.astype(np.float16),
        "x8": _e4m3(xf * S_X),
        "w_phi_t16": w_phi_t16,
        "w_theta_t16": w_theta_t16,
        "w_g16": w_g16,
        "w_mask_t8": w_mask_t8,
    }


def prep_weights(w_phi, w_theta, w_g, w_mask, gamma):
    w_phi_t16 = np.asarray(w_phi, dtype=np.float32).T.astype(np.float16)
    w_theta_t16 = np.asarray(w_theta, dtype=np.float32).T.astype(np.float16)
    w_g16 = np.asarray(w_g, dtype=np.float32).astype(np.float16)
    gamma64 = float(np.asarray(gamma, dtype=np.float32).reshape(-1)[0])
    w_mask_t8 = _e4m3(
        (np.asarray(w_mask, dtype=np.float64).T * gamma64 * S_M).astype(np.float32)
    )
    return w_phi_t16, w_theta_t16, w_g16, w_mask_t8


def kernel(x, w_phi, w_theta, w_g, w_mask, gamma):
    global LAST_RESULT
    x = np.ascontiguousarray(np.asarray(x, dtype=np.float32))
    B, c, h, w = x.shape
    assert (c, h * w) == (C, N), (x.shape,)

    w_phi_t16, w_theta_t16, w_g16, w_mask_t8 = prep_weights(
        w_phi, w_theta, w_g, w_mask, gamma
    )
    nc = get_nc()
    in_maps = [
        make_in_map(x[b], w_phi_t16, w_theta_t16, w_g16, w_mask_t8)
        for b in range(B)
    ]
    trace = bool(int(os.environ.get("KERNEL_TRACE", "0")))
    res = run_bass_kernel_spmd(nc, in_maps, list(range(B)), trace=trace)
    LAST_RESULT = res
    out = np.stack([
        np.asarray(res.results[b]["out"], dtype=np.float32).reshape(c, h, w)
        for b in range(B)
    ])
    return out


# revision 15
# speedup vs baseline: 1.5969x; 1.0028x over previous
# Trainium2 Bass kernel for per-sample channel-attention module (CAM).
#
# Reference math per sample (C=512, N=H*W=4096):
#   X = x.reshape(C, N)
#   phi = Wp X ; theta = Wt X ; g = Wg X
#   attn = softmax_rows(phi @ theta^T)          # [C, C]
#   y = attn @ g                                 # [C, N]
#   Z = (y^T).flatten().reshape(C, N)            # torch permute+view reinterpretation
#   out = gamma * (Wm @ Z) + x
#
# Algebraic restructuring (cuts PE work ~1.8x vs the naive 6-GEMM chain):
#   G = X X^T                  (Gram, [C, C])
#   L = Wp G Wt^T              (attention logits via two small GEMMs)
#   A' = softmax(L) @ Wg       (fold g-projection into attention)
#   y = A' X                   (single big GEMM)
# The torch permute+view reinterpretation is free: y^T blocks are produced
# with a stride-8 column selection of X as the stationary matmul operand, so
# each PSUM tile lands exactly on a contiguous block of Z's SBUF layout.
#
# Mixed precision (validated against the fp64 reference; the softmax here is
# a hard argmax with large top1-top2 logit gaps, so post-softmax stages are
# linear in quantization error while the logit path needs >=10 bits):
#   - logit path (X^T stream, Gram, G, T1, Wp, Wt): fp16 — fp16 weights get
#     the automatic fast-weight-load path so LDWEIGHTS hides behind matmuls
#   - attn: fp16 (fp32 PSUM + exact max-subtraction in the softmax)
#   - A', X, Z, gamma*Wm^T: fp8 e4m3 with power-of-2 scales; ZS and mask
#     GEMMs run in DoubleRow perf mode (K=256 per pass, ~2x fp16 rate)
#   - residual: fp16 x, added via PSUM prefill (scale 2^16) so the final
#     PSUM->SBUF copy is a single scaled cast to the fp16 output
# Gram exploits symmetry: only upper-triangle blocks are computed (1280 of
# 2048 moving columns per tile); the 6 lower blocks come from PE transposes.

import os
import numpy as np
import ml_dtypes

import concourse.bass as bass
import concourse.mybir as mybir
import concourse.tile as tile
from concourse import bacc
from concourse.bass_utils import run_bass_kernel_spmd
from concourse.tile import TileContext

P = 128          # partitions
C = 512          # channels
N = 4096         # spatial (64*64)
CC = C // P      # 4 channel chunks
NT = N // P      # 32 spatial tiles
QF = N // C      # 8 fold factor for the permute+view reinterpretation
FP32 = mybir.dt.float32
FP16 = mybir.dt.float16
FP8 = mybir.dt.float8e4

S_X = 8.0        # x fp8 scale
S_A = 512.0      # A' fp8 scale
S_M = 8192.0     # gamma*Wm^T fp8 scale
S_MX = S_M * S_X           # 65536: residual prefill scale
ZCAST = S_X / (S_A * S_X)  # PSUM (y*S_A*S_X) -> Z*S_X


def _build_nc():
    nc = bacc.Bacc("TRN2", target_bir_lowering=False, debug=False, num_devices=8)
    id_d = nc.dram_tensor("id16", [P, P], FP16, kind="ExternalInput").ap()
    xt_d = nc.dram_tensor("xt16", [N, C], FP16, kind="ExternalInput").ap()
    x16_d = nc.dram_tensor("x16", [C, N], FP16, kind="ExternalInput").ap()
    x8_d = nc.dram_tensor("x8", [C, N], FP8, kind="ExternalInput").ap()
    wphiT_d = nc.dram_tensor("w_phi_t16", [C, C], FP16, kind="ExternalInput").ap()
    wthetaT_d = nc.dram_tensor("w_theta_t16", [C, C], FP16, kind="ExternalInput").ap()
    wg_d = nc.dram_tensor("w_g16", [C, C], FP16, kind="ExternalInput").ap()
    wmT8_d = nc.dram_tensor("w_mask_t8", [C, C], FP8, kind="ExternalInput").ap()
    out_d = nc.dram_tensor("out", [C, N], FP16, kind="ExternalOutput").ap()

    with TileContext(nc) as tc:
        _body(tc, id_d, xt_d, x16_d, x8_d, wphiT_d, wthetaT_d, wg_d, wmT8_d, out_d)
    nc.compile()
    return nc


def _body(tc, id_d, xt_d, x16_d, x8_d, wphiT_d, wthetaT_d, wg_d, wmT8_d, out_d):
    nc = tc.nc
    from contextlib import ExitStack

    with ExitStack() as ctx:
        const = ctx.enter_context(tc.tile_pool(name="const", bufs=1))
        xtp = ctx.enter_context(tc.tile_pool(name="xtp", bufs=1))
        xin = ctx.enter_context(tc.tile_pool(name="xin", bufs=1))
        wpool = ctx.enter_context(tc.tile_pool(name="wpool", bufs=1))
        mid = ctx.enter_context(tc.tile_pool(name="mid", bufs=1))
        vecs = ctx.enter_context(tc.tile_pool(name="vecs", bufs=8))
        outp = ctx.enter_context(tc.tile_pool(name="outp", bufs=4))
        psA = ctx.enter_context(tc.tile_pool(name="psA", bufs=2, space="PSUM"))
        psB = ctx.enter_context(tc.tile_pool(name="psB", bufs=2, space="PSUM"))

        # Warm source: DVE memset (no gpsimd in the startup path). ~18
        # throwaway matmuls ramp the PE p-state while DMA streams in.
        wsrc = const.tile([P, C], FP16)
        nc.vector.memset(wsrc, 1.0)
        warm = psB.tile([P, 2, C], FP32, tag="psB")
        for _ in range(10):
            nc.tensor.matmul(
                warm[:, 0, :], wsrc[:, :P], wsrc, start=True, stop=True
            )

        # ---- input DMA schedule.
        # sync HWDGE:   xt even chunks, x16 left half; output stores later.
        # scalar HWDGE: identity, xt odd chunks, w_theta, w_phi, wg, x16 right.
        # gpsimd SWDGE: x8, wmT8 (needed latest; software queue).
        id16 = const.tile([P, P], FP16)
        xt_sb = xtp.tile([P, NT, C], FP16)
        x16_sb = xin.tile([P, CC, N], FP16)
        x8_sb = xin.tile([P, CC, N], FP8)
        wphiT = wpool.tile([P, CC, C], FP16)
        wthetaT = wpool.tile([P, CC, C], FP16)
        wg16 = wpool.tile([P, CC, C], FP16)
        wmT8 = wpool.tile([P, CC, C], FP8)

        # xt in 8 four-tile chunks round-robined over all three DMA paths
        # (sync HWDGE, scalar HWDGE, gpsimd SWDGE) — per-chunk fixed costs
        # (~1.3us issue+DGE latency) would underfeed the Gram on two queues.
        engs = [nc.sync, nc.scalar, nc.gpsimd]
        NCHUNK = 8
        TPC = NT // NCHUNK
        for k in range(NCHUNK):
            engs[k % 3].dma_start(
                out=xt_sb[:, k * TPC:(k + 1) * TPC, :],
                in_=xt_d[k * TPC * P:(k + 1) * TPC * P, :].rearrange(
                    "(tt p) c -> p tt c", p=P
                ),
            )
        nc.gpsimd.dma_start(out=id16, in_=id_d)
        nc.gpsimd.dma_start(
            out=wthetaT, in_=wthetaT_d.rearrange("(cc p) j -> p cc j", p=P)
        )
        nc.gpsimd.dma_start(
            out=wphiT, in_=wphiT_d.rearrange("(cc p) j -> p cc j", p=P)
        )
        nc.gpsimd.dma_start(
            out=wg16, in_=wg_d.rearrange("(cc p) j -> p cc j", p=P)
        )
        nc.gpsimd.dma_start(
            out=wmT8, in_=wmT8_d.rearrange("(cc p) j -> p cc j", p=P)
        )
        nc.gpsimd.dma_start(
            out=x8_sb, in_=x8_d.rearrange("(cc p) n -> p cc n", p=P)
        )
        HN = N // 2
        nc.sync.dma_start(
            out=x16_sb[:, :, :HN],
            in_=x16_d[:, :HN].rearrange("(cc p) n -> p cc n", p=P),
        )
        nc.scalar.dma_start(
            out=x16_sb[:, :, HN:],
            in_=x16_d[:, HN:].rearrange("(cc p) n -> p cc n", p=P),
        )

        # ---- Gram, upper triangle only: G[mc-block, 128*mc:] accumulated
        # over the 32 streamed xt tiles. gaccA = rows 0,1; gaccB = rows 2,3.
        gaccA = psA.tile([P, 2, C], FP32, tag="psA")
        gaccB = psA.tile([P, 2, C], FP32, tag="psA")
        gacc = [
            (gaccA[:, 0, :], 0), (gaccA[:, 1, :C - P], P),
            (gaccB[:, 0, :C - 2 * P], 2 * P), (gaccB[:, 1, :C - 3 * P], 3 * P),
        ]
        for t in range(NT):
            for mc in range(CC):
                dst, col0 = gacc[mc]
                nc.tensor.matmul(
                    dst,
                    xt_sb[:, t, mc * P:(mc + 1) * P],
                    xt_sb[:, t, col0:],
                    start=(t == 0),
                    stop=(t == NT - 1),
                )

        # full G (fp16) in SBUF: triangle rows + 6 transposed lower blocks
        g_sb = mid.tile([P, CC, C], FP16)
        for mc in range(CC):
            src, col0 = gacc[mc]
            if mc % 2 == 0:
                nc.scalar.activation(
                    out=g_sb[:, mc, col0:], in_=src,
                    func=mybir.ActivationFunctionType.Copy, scale=1.0,
                )
            else:
                nc.vector.tensor_copy(g_sb[:, mc, col0:], src)

        t1_sb = mid.tile([P, CC, C], FP16)
        tps = {}
        tA = psB.tile([P, 2, C], FP32, tag="psB")
        tps[3], tps[2] = tA[:, 0, :], tA[:, 1, :]
        tB = psB.tile([P, 2, C], FP32, tag="psB")
        tps[1], tps[0] = tB[:, 0, :], tB[:, 1, :]

        def t1_pass(mc):
            # T1 = G @ Wt^T (G blocks stationary). mc=3 uses only triangle
            # rows; other mc need the transposed lower blocks.
            tp = tps[mc]
            for jc in range(CC):
                nc.tensor.matmul(
                    tp,
                    g_sb[:, jc, mc * P:(mc + 1) * P],
                    wthetaT[:, jc, :],
                    start=(jc == 0),
                    stop=(jc == CC - 1),
                )
            nc.any.tensor_copy(t1_sb[:, mc, :], tp)

        t1_pass(3)
        # lower blocks (a > b): G[a, b-block] = T(G[b, a-block]); packed into
        # one fp16 PSUM tile, copied back right after each transpose so each
        # T1 pass unblocks as early as possible (T1(mc) needs blocks (*,mc)).
        gt = psA.tile([P, 2, 4, P], FP16, tag="psA")
        lower = [(3, 2), (2, 1), (3, 1), (1, 0), (2, 0), (3, 0)]

        def g_fill(i):
            a, b = lower[i]
            nc.tensor.transpose(
                gt[:, i // 4, i % 4, :],
                g_sb[:, b, a * P:(a + 1) * P],
                id16,
            )
            nc.any.tensor_copy(
                g_sb[:, a, b * P:(b + 1) * P], gt[:, i // 4, i % 4, :]
            )

        g_fill(0)
        t1_pass(2)
        g_fill(1)
        g_fill(2)
        t1_pass(1)
        g_fill(3)
        g_fill(4)
        g_fill(5)
        t1_pass(0)

        # ---- L = Wp @ T1 ; softmax rows -> attn (fp16); attn^T transposes
        # interleaved per-mc so the PE never waits on a softmax.
        attn_sb = mid.tile([P, CC, C], FP16)
        attnT16 = mid.tile([P, CC, C], FP16)
        ptA = psA.tile([P, 2, C], FP16, tag="psA")  # attnT rows dc=0,1
        ptB = psA.tile([P, 2, C], FP16, tag="psA")  # attnT rows dc=2,3
        pt = {0: ptA[:, 0, :], 1: ptA[:, 1, :], 2: ptB[:, 0, :], 3: ptB[:, 1, :]}
        lps = {}

        def l_pass(mc):
            lp = lps[mc]
            for ic in range(CC):
                nc.tensor.matmul(
                    lp,
                    wphiT[:, ic, mc * P:(mc + 1) * P],
                    t1_sb[:, ic, :],
                    start=(ic == 0),
                    stop=(ic == CC - 1),
                )
            neg_max = vecs.tile([P, 1], FP32)
            nc.vector.tensor_reduce(
                out=neg_max, in_=lp, axis=mybir.AxisListType.X,
                op=mybir.AluOpType.max, negate=True,
            )
            sums = vecs.tile([P, 1], FP32)
            nc.scalar.activation(
                out=attn_sb[:, mc, :], in_=lp,
                func=mybir.ActivationFunctionType.Exp,
                bias=neg_max, scale=1.0, accum_out=sums,
            )
            rinv = vecs.tile([P, 1], FP32)
            nc.vector.reciprocal(rinv, sums)
            nc.vector.tensor_scalar_mul(
                attn_sb[:, mc, :], attn_sb[:, mc, :], rinv
            )

        def at_pass(mc):
            for dc in range(CC):
                nc.tensor.transpose(
                    pt[dc][:, mc * P:(mc + 1) * P],
                    attn_sb[:, mc, dc * P:(dc + 1) * P],
                    id16,
                )

        # L order 3,0,1,2: softmax(3) (the attnT straggler otherwise) hides
        # under L(0)/L(1); each at_pass only needs its own softmax done.
        lpA = psB.tile([P, 2, C], FP32, tag="psB")
        lpB = psB.tile([P, 2, C], FP32, tag="psB")
        lps[3], lps[0] = lpA[:, 0, :], lpA[:, 1, :]
        lps[1], lps[2] = lpB[:, 0, :], lpB[:, 1, :]
        l_pass(3)
        l_pass(0)
        l_pass(1)
        at_pass(3)
        at_pass(0)
        l_pass(2)
        at_pass(1)
        at_pass(2)
        for dc in range(CC):
            nc.any.tensor_copy(attnT16[:, dc, :], pt[dc])
        # p-state filler: keep the PE clocked through the attnT-copy bubble
        # so the A'-fold matmuls run at full rate.
        fill = psB.tile([P, 2, C], FP32, tag="psB")
        for _ in range(6):
            nc.tensor.matmul(
                fill[:, 0, :], wsrc[:, :P], wsrc, start=True, stop=True
            )

        # ---- A'^T[j, c] = sum_d Wg[d, j] attn[c, d]; cast to fp8 * S_A
        apT8 = mid.tile([P, CC, C], FP8)
        for half in range(2):
            ap_ps = psB.tile([P, 2, C], FP32, tag="psB")
            for sub in range(2):
                jc = half * 2 + sub
                for dc in range(CC):
                    nc.tensor.matmul(
                        ap_ps[:, sub, :],
                        wg16[:, dc, jc * P:(jc + 1) * P],
                        attnT16[:, dc, :],
                        start=(dc == 0),
                        stop=(dc == CC - 1),
                    )
                nc.scalar.activation(
                    out=apT8[:, jc, :], in_=ap_ps[:, sub, :],
                    func=mybir.ActivationFunctionType.Copy, scale=S_A,
                )

        # ---- ZS (y^T blocks in Z layout) + mask GEMM + residual + store,
        # both in fp8 DoubleRow (K=256 per pass). Software-pipelined:
        # mask(q-1) runs on PE while ZS(q)'s PSUM->fp8 casts run on ACT/DVE.
        zs8 = mid.tile([P, CC, N], FP8)
        xr8 = x8_sb.rearrange("p cc (ci m q) -> p cc ci q m", ci=CC, q=QF)

        def zs_half(q, ci2):
            # 4 DoubleRow matmuls -> one ACT cast to fp8 (Z * S_X)
            pool = psA if ci2 == 0 else psB
            zp = pool.tile([P, 2, C], FP32, tag="psA" if ci2 == 0 else "psB")
            for s in range(2):
                ci = ci2 * 2 + s
                for j2 in range(2):
                    nc.tensor.matmul(
                        zp[:, s, :],
                        xr8[:, 2 * j2:2 * j2 + 2, ci, q, :],
                        apT8[:, 2 * j2:2 * j2 + 2, :],
                        start=(j2 == 0),
                        stop=(j2 == 1),
                        perf_mode=mybir.MatmulPerfMode.DoubleRow,
                    )
            nc.scalar.activation(
                out=zs8[:, 2 * ci2:2 * ci2 + 2, q * C:(q + 1) * C],
                in_=zp, func=mybir.ActivationFunctionType.Copy,
                scale=ZCAST,
            )

        def mask_half(q, oc2):
            # 4 DoubleRow matmuls (PSUM = mask * S_MX), then one fused DVE
            # op: out = PSUM/S_MX + x (residual), straight to fp16.
            pool = psA if oc2 == 0 else psB
            mp = pool.tile([P, 2, C], FP32, tag="psA" if oc2 == 0 else "psB")
            for s in range(2):
                oc = oc2 * 2 + s
                for i2 in range(2):
                    nc.tensor.matmul(
                        mp[:, s, :],
                        wmT8[:, 2 * i2:2 * i2 + 2, oc * P:(oc + 1) * P],
                        zs8[:, 2 * i2:2 * i2 + 2, q * C:(q + 1) * C],
                        start=(i2 == 0),
                        stop=(i2 == 1),
                        perf_mode=mybir.MatmulPerfMode.DoubleRow,
                    )
            ot = outp.tile([P, 2, C], FP16)
            nc.vector.scalar_tensor_tensor(
                out=ot, in0=mp, scalar=1.0 / S_MX,
                in1=x16_sb[:, 2 * oc2:2 * oc2 + 2, q * C:(q + 1) * C],
                op0=mybir.AluOpType.mult, op1=mybir.AluOpType.add,
            )
            nc.sync.dma_start(
                out=out_d[
                    oc2 * 2 * P:(oc2 * 2 + 2) * P, q * C:(q + 1) * C
                ].rearrange("(cc p) n -> p cc n", p=P),
                in_=ot,
            )

        for q in range(QF):
            zs_half(q, 0)
            if q > 0:
                mask_half(q - 1, 0)
            zs_half(q, 1)
            if q > 0:
                mask_half(q - 1, 1)
        mask_half(QF - 1, 0)
        mask_half(QF - 1, 1)


_NC_CACHE = {}
LAST_RESULT = None


def get_nc():
    if "nc" not in _NC_CACHE:
        _NC_CACHE["nc"] = _build_nc()
    return _NC_CACHE["nc"]


def _e4m3(a):
    return np.asarray(
        np.clip(np.asarray(a, np.float32), -448.0, 448.0),
        ml_dtypes.float8_e4m3fn,
    )


_ID16 = np.eye(P, dtype=np.float16)


def make_in_map(xb, w_phi_t16, w_theta_t16, w_g16, w_mask_t8):
    """Per-core input dict; xb is one sample [C, H, W]."""
    xf = np.ascontiguousarray(xb.reshape(C, N), dtype=np.float32)
    return {
        "id16": _ID16,
        "xt16": np.ascontiguousarray(xf.T).astype(np.float16),
        "x16": xf.astype(np.float16),
        "x8": _e4m3(xf * S_X),
        "w_phi_t16": w_phi_t16,
        "w_theta_t16": w_theta_t16,
        "w_g16": w_g16,
        "w_mask_t8": w_mask_t8,
    }


def prep_weights(w_phi, w_theta, w_g, w_mask, gamma):
    w_phi_t16 = np.asarray(w_phi, dtype=np.float32).T.astype(np.float16)
    w_theta_t16 = np.asarray(w_theta, dtype=np.float32).T.astype(np.float16)
    w_g16 = np.asarray(w_g, dtype=np.float32).astype(np.float16)
    gamma64 = float(np.asarray(gamma, dtype=np.float32).reshape(-1)[0])
    w_mask_t8 = _e4m3(
        (np.asarray(w_mask, dtype=np.float64).T * gamma64 * S_M).astype(np.float32)
    )
    return w_phi_t16, w_theta_t16, w_g16, w_mask_t8


def kernel(x, w_phi, w_theta, w_g, w_mask, gamma):
    global LAST_RESULT
    x = np.ascontiguousarray(np.asarray(x, dtype=np.float32))
    B, c, h, w = x.shape
    assert (c, h * w) == (C, N), (x.shape,)

    w_phi_t16, w_theta_t16, w_g16, w_mask_t8 = prep_weights(
        w_phi, w_theta, w_g, w_mask, gamma
    )
    nc = get_nc()
    in_maps = [
        make_in_map(x[b], w_phi_t16, w_theta_t16, w_g16, w_mask_t8)
        for b in range(B)
    ]
    trace = bool(int(os.environ.get("KERNEL_TRACE", "0")))
    res = run_bass_kernel_spmd(nc, in_maps, list(range(B)), trace=trace)
    LAST_RESULT = res
    out = np.stack([
        np.asarray(res.results[b]["out"], dtype=np.float32).reshape(c, h, w)
        for b in range(B)
    ])
    return out


# revision 22
# speedup vs baseline: 1.8457x; 1.1558x over previous
# Trainium2 Bass kernel for per-sample channel-attention module (CAM).
#
# Reference math per sample (C=512, N=H*W=4096):
#   X = x.reshape(C, N)
#   phi = Wp X ; theta = Wt X ; g = Wg X
#   attn = softmax_rows(phi @ theta^T)          # [C, C]
#   y = attn @ g                                 # [C, N]
#   Z = (y^T).flatten().reshape(C, N)            # torch permute+view reinterpretation
#   out = gamma * (Wm @ Z) + x
#
# Algebraic restructuring (cuts PE work ~1.8x vs the naive 6-GEMM chain):
#   G = X X^T                  (Gram, [C, C])
#   L = Wp G Wt^T              (attention logits via two small GEMMs)
#   A' = softmax(L) @ Wg       (fold g-projection into attention)
#   y = A' X                   (single big GEMM)
# The torch permute+view reinterpretation is free: y^T blocks are produced
# with a stride-8 column selection of X as the stationary matmul operand, so
# each PSUM tile lands exactly on a contiguous block of Z's SBUF layout.
#
# Mixed precision (validated against the fp64 reference; the softmax here is
# a hard argmax with large top1-top2 logit gaps, so post-softmax stages are
# linear in quantization error while the logit path needs >=10 bits):
#   - logit path (X^T stream, Gram, G, T1, Wp, Wt): fp16 — fp16 weights get
#     the automatic fast-weight-load path so LDWEIGHTS hides behind matmuls
#   - attn: fp16 (fp32 PSUM + exact max-subtraction in the softmax)
#   - A', X, Z, gamma*Wm^T: fp8 e4m3 with power-of-2 scales; ZS and mask
#     GEMMs run in DoubleRow perf mode (K=256 per pass, ~2x fp16 rate)
#   - residual: fp16 x, added via PSUM prefill (scale 2^16) so the final
#     PSUM->SBUF copy is a single scaled cast to the fp16 output
# Gram exploits symmetry: only upper-triangle blocks are computed (1280 of
# 2048 moving columns per tile); the 6 lower blocks come from PE transposes.

import os
import numpy as np
import ml_dtypes

import concourse.bass as bass
import concourse.mybir as mybir
import concourse.tile as tile
from concourse import bacc
from concourse.bass_utils import run_bass_kernel_spmd
from concourse.tile import TileContext

P = 128          # partitions
C = 512          # channels
N = 4096         # spatial (64*64)
CC = C // P      # 4 channel chunks
NT = N // P      # 32 spatial tiles
QF = N // C      # 8 fold factor for the permute+view reinterpretation
FP32 = mybir.dt.float32
FP16 = mybir.dt.float16
FP8 = mybir.dt.float8e4

S_X = 8.0        # x fp8 scale
S_A = 512.0      # A' fp8 scale
S_M = 8192.0     # gamma*Wm^T fp8 scale
S_MX = S_M * S_X           # 65536: residual prefill scale
ZCAST = S_X / (S_A * S_X)  # PSUM (y*S_A*S_X) -> Z*S_X


def _build_nc():
    # All DRAM tensors are HOST-PACKED into the exact SBUF layout
    # ([128 partitions, flat free dim]) so every DMA descriptor moves a
    # 2-16KB contiguous run — 1KB fp16 rows were descriptor-rate-bound.
    nc = bacc.Bacc("TRN2", target_bir_lowering=False, debug=False, num_devices=8)
    id_d = nc.dram_tensor("id16", [P, P], FP16, kind="ExternalInput").ap()
    xt_d = nc.dram_tensor("xt16p", [P, NT * C], FP16, kind="ExternalInput").ap()
    x16_d = nc.dram_tensor("x16p", [P, CC * N], FP16, kind="ExternalInput").ap()
    x8_d = nc.dram_tensor("x8p", [P, CC * N], FP8, kind="ExternalInput").ap()
    wphiT_d = nc.dram_tensor("w_phi_tp", [P, CC * C], FP16, kind="ExternalInput").ap()
    wthetaT_d = nc.dram_tensor("w_theta_tp", [P, CC * C], FP16, kind="ExternalInput").ap()
    wg_d = nc.dram_tensor("w_gp", [P, CC * C], FP16, kind="ExternalInput").ap()
    wmT8_d = nc.dram_tensor("w_mask_tp8", [P, CC * C], FP8, kind="ExternalInput").ap()
    # out rows: (q, oc2, s, c) per partition; host unpacks to [C, N]
    out_d = nc.dram_tensor("outp", [P, QF * 2 * 2 * C], FP16, kind="ExternalOutput").ap()

    with TileContext(nc) as tc:
        _body(tc, id_d, xt_d, x16_d, x8_d, wphiT_d, wthetaT_d, wg_d, wmT8_d, out_d)
    nc.compile()
    return nc


def _body(tc, id_d, xt_d, x16_d, x8_d, wphiT_d, wthetaT_d, wg_d, wmT8_d, out_d):
    nc = tc.nc
    from contextlib import ExitStack

    with ExitStack() as ctx:
        const = ctx.enter_context(tc.tile_pool(name="const", bufs=1))
        xtp = ctx.enter_context(tc.tile_pool(name="xtp", bufs=1))
        xin = ctx.enter_context(tc.tile_pool(name="xin", bufs=1))
        wpool = ctx.enter_context(tc.tile_pool(name="wpool", bufs=1))
        mid = ctx.enter_context(tc.tile_pool(name="mid", bufs=1))
        vecs = ctx.enter_context(tc.tile_pool(name="vecs", bufs=8))
        outp = ctx.enter_context(tc.tile_pool(name="outp", bufs=4))
        psA = ctx.enter_context(tc.tile_pool(name="psA", bufs=2, space="PSUM"))
        psB = ctx.enter_context(tc.tile_pool(name="psB", bufs=2, space="PSUM"))

        # Warm source: DVE memset (no gpsimd in the startup path). ~18
        # throwaway matmuls ramp the PE p-state while DMA streams in.
        wsrc = const.tile([P, C], FP16)
        nc.vector.memset(wsrc, 1.0)
        warm = psB.tile([P, 2, C], FP32, tag="psB")
        for _ in range(10):
            nc.tensor.matmul(
                warm[:, 0, :], wsrc[:, :P], wsrc, start=True, stop=True
            )

        # ---- input DMA schedule.
        # sync HWDGE:   xt even chunks, x16 left half; output stores later.
        # scalar HWDGE: identity, xt odd chunks, w_theta, w_phi, wg, x16 right.
        # gpsimd SWDGE: x8, wmT8 (needed latest; software queue).
        id16 = const.tile([P, P], FP16)
        xt_sb = xtp.tile([P, NT, C], FP16)
        x16_sb = xin.tile([P, CC, N], FP16)
        x8_sb = xin.tile([P, CC, N], FP8)
        wphiT = wpool.tile([P, CC, C], FP16)
        wthetaT = wpool.tile([P, CC, C], FP16)
        wg16 = wpool.tile([P, CC, C], FP16)
        wmT8 = wpool.tile([P, CC, C], FP8)

        # xt in 8 four-tile chunks (4KB/partition each) alternating the two
        # HWDGE queues; weights/identity/x8 on the SWDGE path.
        NCHUNK = 8
        TPC = NT // NCHUNK
        CW = TPC * C
        for k in range(NCHUNK):
            eng = nc.sync if (k % 2 == 0) else nc.scalar
            eng.dma_start(
                out=xt_sb[:, k * TPC:(k + 1) * TPC, :],
                in_=xt_d[:, k * CW:(k + 1) * CW],
            )
        nc.gpsimd.dma_start(out=id16, in_=id_d)
        nc.gpsimd.dma_start(out=wthetaT, in_=wthetaT_d)
        nc.gpsimd.dma_start(out=wphiT, in_=wphiT_d)
        nc.gpsimd.dma_start(out=wg16, in_=wg_d)
        nc.gpsimd.dma_start(out=wmT8, in_=wmT8_d)
        nc.gpsimd.dma_start(out=x8_sb, in_=x8_d)
        HN = N // 2
        xv = x16_d.rearrange("p (cc n) -> p cc n", cc=CC)
        nc.sync.dma_start(out=x16_sb[:, :, :HN], in_=xv[:, :, :HN])
        nc.scalar.dma_start(out=x16_sb[:, :, HN:], in_=xv[:, :, HN:])

        # ---- Gram, upper triangle only: G[mc-block, 128*mc:] accumulated
        # over the 32 streamed xt tiles. gaccA = rows 0,1; gaccB = rows 2,3.
        gaccA = psA.tile([P, 2, C], FP32, tag="psA")
        gaccB = psA.tile([P, 2, C], FP32, tag="psA")
        gacc = [
            (gaccA[:, 0, :], 0), (gaccA[:, 1, :C - P], P),
            (gaccB[:, 0, :C - 2 * P], 2 * P), (gaccB[:, 1, :C - 3 * P], 3 * P),
        ]
        for t in range(NT):
            for mc in range(CC):
                dst, col0 = gacc[mc]
                nc.tensor.matmul(
                    dst,
                    xt_sb[:, t, mc * P:(mc + 1) * P],
                    xt_sb[:, t, col0:],
                    start=(t == 0),
                    stop=(t == NT - 1),
                )

        # full G (fp16) in SBUF: triangle rows + 6 transposed lower blocks
        g_sb = mid.tile([P, CC, C], FP16)
        for mc in range(CC):
            src, col0 = gacc[mc]
            if mc % 2 == 0:
                nc.scalar.activation(
                    out=g_sb[:, mc, col0:], in_=src,
                    func=mybir.ActivationFunctionType.Copy, scale=1.0,
                )
            else:
                nc.vector.tensor_copy(g_sb[:, mc, col0:], src)

        t1_sb = mid.tile([P, CC, C], FP16)
        tps = {}
        tA = psB.tile([P, 2, C], FP32, tag="psB")
        tps[3], tps[2] = tA[:, 0, :], tA[:, 1, :]
        tB = psB.tile([P, 2, C], FP32, tag="psB")
        tps[1], tps[0] = tB[:, 0, :], tB[:, 1, :]

        def t1_pass(mc):
            # T1 = G @ Wt^T (G blocks stationary). mc=3 uses only triangle
            # rows; other mc need the transposed lower blocks.
            tp = tps[mc]
            for jc in range(CC):
                nc.tensor.matmul(
                    tp,
                    g_sb[:, jc, mc * P:(mc + 1) * P],
                    wthetaT[:, jc, :],
                    start=(jc == 0),
                    stop=(jc == CC - 1),
                )
            nc.any.tensor_copy(t1_sb[:, mc, :], tp)

        t1_pass(3)
        # lower blocks (a > b): G[a, b-block] = T(G[b, a-block]); packed into
        # one fp16 PSUM tile, copied back right after each transpose so each
        # T1 pass unblocks as early as possible (T1(mc) needs blocks (*,mc)).
        gt = psA.tile([P, 2, 4, P], FP16, tag="psA")
        lower = [(3, 2), (2, 1), (3, 1), (1, 0), (2, 0), (3, 0)]

        def g_fill(i):
            a, b = lower[i]
            nc.tensor.transpose(
                gt[:, i // 4, i % 4, :],
                g_sb[:, b, a * P:(a + 1) * P],
                id16,
            )
            nc.any.tensor_copy(
                g_sb[:, a, b * P:(b + 1) * P], gt[:, i // 4, i % 4, :]
            )

        g_fill(0)
        t1_pass(2)
        g_fill(1)
        g_fill(2)
        t1_pass(1)
        g_fill(3)
        g_fill(4)
        g_fill(5)
        t1_pass(0)

        # ---- L = Wp @ T1 ; softmax rows -> attn (fp16); attn^T transposes
        # interleaved per-mc so the PE never waits on a softmax.
        attn_sb = mid.tile([P, CC, C], FP16)
        attnT16 = mid.tile([P, CC, C], FP16)
        ptA = psA.tile([P, 2, C], FP16, tag="psA")  # attnT rows dc=0,1
        ptB = psA.tile([P, 2, C], FP16, tag="psA")  # attnT rows dc=2,3
        pt = {0: ptA[:, 0, :], 1: ptA[:, 1, :], 2: ptB[:, 0, :], 3: ptB[:, 1, :]}
        lps = {}

        def l_pass(mc):
            lp = lps[mc]
            # descending ic: the first matmuls use the T1 rows copied
            # earliest (T1 runs mc=3..0), so L never waits on the last copy
            for ic in reversed(range(CC)):
                nc.tensor.matmul(
                    lp,
                    wphiT[:, ic, mc * P:(mc + 1) * P],
                    t1_sb[:, ic, :],
                    start=(ic == CC - 1),
                    stop=(ic == 0),
                )
            neg_max = vecs.tile([P, 1], FP32)
            nc.vector.tensor_reduce(
                out=neg_max, in_=lp, axis=mybir.AxisListType.X,
                op=mybir.AluOpType.max, negate=True,
            )
            sums = vecs.tile([P, 1], FP32)
            nc.scalar.activation(
                out=attn_sb[:, mc, :], in_=lp,
                func=mybir.ActivationFunctionType.Exp,
                bias=neg_max, scale=1.0, accum_out=sums,
            )
            rinv = vecs.tile([P, 1], FP32)
            nc.vector.reciprocal(rinv, sums)
            nc.vector.tensor_scalar_mul(
                attn_sb[:, mc, :], attn_sb[:, mc, :], rinv
            )

        def at_pass(mc):
            for dc in range(CC):
                nc.tensor.transpose(
                    pt[dc][:, mc * P:(mc + 1) * P],
                    attn_sb[:, mc, dc * P:(dc + 1) * P],
                    id16,
                )

        # L order 3,0,1,2: softmax(3) (the attnT straggler otherwise) hides
        # under L(0)/L(1); each at_pass only needs its own softmax done.
        lpA = psB.tile([P, 2, C], FP32, tag="psB")
        lpB = psB.tile([P, 2, C], FP32, tag="psB")
        lps[3], lps[0] = lpA[:, 0, :], lpA[:, 1, :]
        lps[1], lps[2] = lpB[:, 0, :], lpB[:, 1, :]
        l_pass(3)
        l_pass(0)
        l_pass(1)
        at_pass(3)
        at_pass(0)
        l_pass(2)
        at_pass(1)
        at_pass(2)
        for dc in range(CC):
            nc.any.tensor_copy(attnT16[:, dc, :], pt[dc])
        # p-state filler: keep the PE clocked through the attnT-copy bubble
        # so the A'-fold matmuls run at full rate. Reuses lps[1]'s PSUM slice
        # (its softmax reads finished long ago) to avoid any WAR stall.
        for _ in range(6):
            nc.tensor.matmul(
                lps[1], wsrc[:, :P], wsrc, start=True, stop=True
            )

        # ---- A'^T[j, c] = sum_d Wg[d, j] attn[c, d]; cast to fp8 * S_A
        apT8 = mid.tile([P, CC, C], FP8)
        for half in range(2):
            ap_ps = psB.tile([P, 2, C], FP32, tag="psB")
            for sub in range(2):
                jc = half * 2 + sub
                for dc in range(CC):
                    nc.tensor.matmul(
                        ap_ps[:, sub, :],
                        wg16[:, dc, jc * P:(jc + 1) * P],
                        attnT16[:, dc, :],
                        start=(dc == 0),
                        stop=(dc == CC - 1),
                    )
                nc.scalar.activation(
                    out=apT8[:, jc, :], in_=ap_ps[:, sub, :],
                    func=mybir.ActivationFunctionType.Copy, scale=S_A,
                )

        # ---- ZS (y^T blocks in Z layout) + mask GEMM + residual + store,
        # both in fp8 DoubleRow (K=256 per pass). Software-pipelined:
        # mask(q-1) runs on PE while ZS(q)'s PSUM->fp8 casts run on ACT/DVE.
        zs8 = mid.tile([P, CC, N], FP8)
        xr8 = x8_sb.rearrange("p cc (ci m q) -> p cc ci q m", ci=CC, q=QF)

        def zs_half(q, ci2):
            # 4 DoubleRow matmuls -> one ACT cast to fp8 (Z * S_X)
            pool = psA if ci2 == 0 else psB
            zp = pool.tile([P, 2, C], FP32, tag="psA" if ci2 == 0 else "psB")
            for s in range(2):
                ci = ci2 * 2 + s
                for j2 in range(2):
                    nc.tensor.matmul(
                        zp[:, s, :],
                        xr8[:, 2 * j2:2 * j2 + 2, ci, q, :],
                        apT8[:, 2 * j2:2 * j2 + 2, :],
                        start=(j2 == 0),
                        stop=(j2 == 1),
                        perf_mode=mybir.MatmulPerfMode.DoubleRow,
                    )
            nc.scalar.activation(
                out=zs8[:, 2 * ci2:2 * ci2 + 2, q * C:(q + 1) * C],
                in_=zp, func=mybir.ActivationFunctionType.Copy,
                scale=ZCAST,
            )

        def mask_half(q, oc2):
            # 4 DoubleRow matmuls (PSUM = mask * S_MX), then one fused DVE
            # op: out = PSUM/S_MX + x (residual), straight to fp16.
            pool = psA if oc2 == 0 else psB
            mp = pool.tile([P, 2, C], FP32, tag="psA" if oc2 == 0 else "psB")
            for s in range(2):
                oc = oc2 * 2 + s
                for i2 in range(2):
                    nc.tensor.matmul(
                        mp[:, s, :],
                        wmT8[:, 2 * i2:2 * i2 + 2, oc * P:(oc + 1) * P],
                        zs8[:, 2 * i2:2 * i2 + 2, q * C:(q + 1) * C],
                        start=(i2 == 0),
                        stop=(i2 == 1),
                        perf_mode=mybir.MatmulPerfMode.DoubleRow,
                    )
            ot = outp.tile([P, 2, C], FP16)
            nc.vector.scalar_tensor_tensor(
                out=ot, in0=mp, scalar=1.0 / S_MX,
                in1=x16_sb[:, 2 * oc2:2 * oc2 + 2, q * C:(q + 1) * C],
                op0=mybir.AluOpType.mult, op1=mybir.AluOpType.add,
            )
            blk = (q * 2 + oc2) * 2 * C
            nc.sync.dma_start(out=out_d[:, blk:blk + 2 * C], in_=ot)

        for q in range(QF):
            zs_half(q, 0)
            if q > 0:
                mask_half(q - 1, 0)
            zs_half(q, 1)
            if q > 0:
                mask_half(q - 1, 1)
        mask_half(QF - 1, 0)
        mask_half(QF - 1, 1)


_NC_CACHE = {}
LAST_RESULT = None


def get_nc():
    if "nc" not in _NC_CACHE:
        _NC_CACHE["nc"] = _build_nc()
    return _NC_CACHE["nc"]


def _e4m3(a):
    return np.asarray(
        np.clip(np.asarray(a, np.float32), -448.0, 448.0),
        ml_dtypes.float8_e4m3fn,
    )


_ID16 = np.eye(P, dtype=np.float16)


def _pack(arr2d):
    """[cc*128, W] -> [128, cc*W]: per-partition-contiguous SBUF layout."""
    a = np.asarray(arr2d)
    cc = a.shape[0] // P
    return np.ascontiguousarray(
        a.reshape(cc, P, a.shape[1]).transpose(1, 0, 2).reshape(P, -1)
    )


def make_in_map(xb, w_phi_tp, w_theta_tp, w_gp, w_mask_tp8):
    """Per-core input dict; xb is one sample [C, H, W]."""
    xf = np.ascontiguousarray(xb.reshape(C, N), dtype=np.float32)
    return {
        "id16": _ID16,
        "xt16p": _pack(np.ascontiguousarray(xf.T).astype(np.float16)),
        "x16p": _pack(xf.astype(np.float16)),
        "x8p": _pack(_e4m3(xf * S_X)),
        "w_phi_tp": w_phi_tp,
        "w_theta_tp": w_theta_tp,
        "w_gp": w_gp,
        "w_mask_tp8": w_mask_tp8,
    }


def prep_weights(w_phi, w_theta, w_g, w_mask, gamma):
    w_phi_tp = _pack(np.asarray(w_phi, dtype=np.float32).T.astype(np.float16))
    w_theta_tp = _pack(np.asarray(w_theta, dtype=np.float32).T.astype(np.float16))
    w_gp = _pack(np.asarray(w_g, dtype=np.float32).astype(np.float16))
    gamma64 = float(np.asarray(gamma, dtype=np.float32).reshape(-1)[0])
    w_mask_tp8 = _pack(_e4m3(
        (np.asarray(w_mask, dtype=np.float64).T * gamma64 * S_M).astype(np.float32)
    ))
    return w_phi_tp, w_theta_tp, w_gp, w_mask_tp8


def kernel(x, w_phi, w_theta, w_g, w_mask, gamma):
    global LAST_RESULT
    x = np.ascontiguousarray(np.asarray(x, dtype=np.float32))
    B, c, h, w = x.shape
    assert (c, h * w) == (C, N), (x.shape,)

    w_phi_tp, w_theta_tp, w_gp, w_mask_tp8 = prep_weights(
        w_phi, w_theta, w_g, w_mask, gamma
    )
    nc = get_nc()
    in_maps = [
        make_in_map(x[b], w_phi_tp, w_theta_tp, w_gp, w_mask_tp8)
        for b in range(B)
    ]
    trace = bool(int(os.environ.get("KERNEL_TRACE", "0")))
    res = run_bass_kernel_spmd(nc, in_maps, list(range(B)), trace=trace)
    LAST_RESULT = res
    out = np.empty((B, c, h * w), dtype=np.float32)
    for b in range(B):
        a = np.asarray(res.results[b]["outp"], dtype=np.float32)
        # [p, (q, oc2, s, c)] -> channel = (2*oc2+s)*128+p, col = q*512+c
        out[b] = a.reshape(P, QF, 2, 2, C).transpose(2, 3, 0, 1, 4).reshape(c, h * w)
    return out.reshape(B, c, h, w)
